# revision 1
# baseline (speedup 1.0000x reference)
"""Trainium2 Bass kernel for nn_NewAttention (analytic Gaussian sparse attention).

Math (per batch element b):
    v        = x[b] @ W_in.T                      # [L, E]
    per head h (P=128 cols of v):
        A_h  = softmax(-(j - c_h(i))^2 / 2)       # [L, L], analytic, banded
        att_h = A_h @ v_h                         # [L, P]
    out[b]   = concat_h(att_h) @ W_out.T          # [L, E]

Sharding: data-parallel over batch, one batch element per NeuronCore (8 cores).

Device strategy (per core):
  - host pre-transposes x[b] -> xT [E, L] so matmul1 needs no on-chip transpose
  - matmul1: v[l-tile, m] = xT-slice.T @ W_inT (stationary = xT 128x128 slices,
    moving = W_inT 512-chunks) -> v in natural layout, resident in SBUF.
  - attention: att^T_h = v_h.T @ A_h^T computed as banded matmuls: stationary =
    v 128x128 slices (contraction over sequence), moving = analytic A^T blocks
    [128, 256] (host-precomputed exact softmax weights; only 7 distinct blocks
    per head type thanks to shift invariance). Each v-tile's band covers a
    q-window padded to N=256 (full-rate float32r) and PSUM accumulates
    overlapping windows via the per-element has_written bits; the first matmul
    touching each PSUM bank uses start=True (whole-bank has_written clear).
    Output is feature-major att^T, exactly what matmul2 needs as stationary.
  - 'first'/'last' heads attend to a fixed key location for every query, so
    their output contribution is a rank-1 update r[e] = (w_h @ v_h) @ W_outT_h,
    broadcast across partitions once (K=1 matmul) and added by the DVE during
    the PSUM->SBUF copy of each output tile. Their v columns are only needed
    at the sequence boundary, so matmul1 skips them for interior tiles.
  - matmul2: out[l-tile, e] = att^T-slice.T @ W_outT chunks -> natural layout,
    contiguous DMA out.

All matmuls run in float32r (TF32-like full-rate fp32: 1 cyc/row at moving
dim >= 256). Measured end-to-end relative error ~2.3e-4.
"""

import sys
import numpy as np

for _p in ("/opt/trn_rl_repo",):
    if _p not in sys.path:
        sys.path.insert(0, _p)

import concourse.bass as bass
import concourse.bacc as bacc
import concourse.mybir as mybir
from concourse import tile
from concourse import bass2jax as _b2j

# ---------------- problem constants (hardcoded per contract) ----------------
B = 8
L = 2048
E = 1024
H = 8
P = 128
SIGMA = 1.0
DISP = 1
NT = L // 128          # 16 l-tiles
KT = E // 128          # 8 k-tiles
NSLAB = 2              # attention q-slabs of 1024
DT = mybir.dt.float32
MM_DT = mybir.dt.float32r

BANDED_HEADS = [0, 1, 2, 5, 6, 7]   # center,left,right,center,left,right
NBH = len(BANDED_HEADS)
HEAD_TYPE = {0: 0, 1: 1, 2: 2, 5: 0, 6: 1, 7: 2}  # 0=center,1=left,2=right
TYPE_DISP = [0, -DISP, +DISP]

# attention A^T block classes: (name, representative tile i0, start_rel)
# window for tile i, class c = [128*i + start_rel, 128*i + start_rel + 256)
CLS = [
    ("int8", 1, -8),      # interior single-window tiles (i%4 in {1,2})
    ("intA", 3, -128),    # i%4==3 piece A
    ("intB", 3, +128),    # i%4==3 piece B
    ("prevA", 4, -256),   # i%4==0 (i>0) piece A
    ("int0", 4, 0),       # i%4==0 (i>0) piece B
    ("first0", 0, 0),     # i==0 (boundary-renormalized rows)
    ("last", 15, -128),   # i==15 (boundary-renormalized rows)
]
CLS_IDX = {name: k for k, (name, _, _) in enumerate(CLS)}
NCLS = len(CLS)


def _pieces(i):
    """A^T matmul pieces for v-tile i: list of (start_rel, class_idx)."""
    if i == 0:
        return [(0, CLS_IDX["first0"])]
    if i == NT - 1:
        return [(-128, CLS_IDX["last"])]
    m = i % 4
    if m in (1, 2):
        return [(-8, CLS_IDX["int8"])]
    if m == 3:
        return [(-128, CLS_IDX["intA"]), (+128, CLS_IDX["intB"])]
    return [(-256, CLS_IDX["prevA"]), (0, CLS_IDX["int0"])]


def _softmax_rows(logits):
    m = logits.max(axis=-1, keepdims=True)
    e = np.exp(logits - m)
    return e / e.sum(axis=-1, keepdims=True)


def _host_tables():
    """Analytic attention weight blocks (exact, float64 -> fp32).

    a_all[p, (t*NCLS + c)*256 + q] = A_t[q0 + q, 128*i0 + p]
    where (i0, start_rel) come from CLS[c] and q0 = 128*i0 + start_rel
    (rows outside [0, L) are zero; none occur by construction).
    """
    j = np.arange(L, dtype=np.float64)
    i = np.arange(L, dtype=np.float64)

    a_all = np.zeros((128, 3 * NCLS * 256), dtype=np.float64)
    for t, disp in enumerate(TYPE_DISP):
        c = i + disp
        logits = -((j[None, :] - c[:, None]) ** 2) / (2.0 * SIGMA**2)
        A = _softmax_rows(logits)  # [Lq, Lk]
        for ci, (_, i0, start_rel) in enumerate(CLS):
            q0 = 128 * i0 + start_rel
            assert 0 <= q0 and q0 + 256 <= L, (i0, start_rel)
            blkcol = (t * NCLS + ci) * 256
            a_all[:, blkcol : blkcol + 256] = A[
                q0 : q0 + 256, 128 * i0 : 128 * i0 + 128
            ].T

    # first/last heads: fixed weight vector over keys (same for every query)
    Af = _softmax_rows(-((j[None, :] - np.zeros((1, 1))) ** 2) / (2 * SIGMA**2))
    Al = _softmax_rows(
        -((j[None, :] - np.full((1, 1), L - 1.0)) ** 2) / (2 * SIGMA**2)
    )
    wfl = np.zeros((128, 2), dtype=np.float64)
    wfl[:, 0] = Af[0, 0:128]         # support at k < 128  (v tile 0)
    wfl[:, 1] = Al[0, L - 128 : L]   # support at k >= L-128 (v tile 15)

    return a_all.astype(np.float32), wfl.astype(np.float32)


def _build_program(phases=3):
    nc = bacc.Bacc("TRN2", target_bir_lowering=False, debug=False, num_devices=B)

    xT = nc.dram_tensor("xT", [E, L], MM_DT, kind="ExternalInput")
    w_inT = nc.dram_tensor("w_inT", [E, E], MM_DT, kind="ExternalInput")
    w_outT = nc.dram_tensor("w_outT", [E, E], MM_DT, kind="ExternalInput")
    a_all = nc.dram_tensor(
        "a_all", [128, 3 * NCLS * 256], MM_DT, kind="ExternalInput"
    )
    wfl = nc.dram_tensor("wfl", [128, 2], DT, kind="ExternalInput")
    ones = nc.dram_tensor("ones", [1, 128], MM_DT, kind="ExternalInput")
    out = nc.dram_tensor("out", [L, E], DT, kind="ExternalOutput")

    with tile.TileContext(nc) as tc:
        with (
            tc.tile_pool(name="const", bufs=1) as cpool,
            tc.tile_pool(name="vbuf", bufs=1) as vpool,
            tc.tile_pool(name="outp", bufs=2) as outpool,
            tc.tile_pool(name="ps_big", bufs=2, space="PSUM") as ps_big,
            tc.tile_pool(name="ps_att", bufs=2, space="PSUM") as ps_att,
        ):
            # resident through phase 2
            w_outT_sb = cpool.tile([128, KT * E], MM_DT, tag="w_outT_sb")
            a_sb = cpool.tile([128, 3 * NCLS * 256], MM_DT, tag="a_sb")
            wfl_sb = cpool.tile([128, 2], DT, tag="wfl_sb")
            ones_sb = cpool.tile([1, 128], MM_DT, tag="ones_sb")
            v_sb = vpool.tile([128, NT * E], MM_DT, tag="v_sb")

            def ablk(t, ci):
                s = (t * NCLS + ci) * 256
                return a_sb[:, s : s + 256]

            # ---- phase 1: v[l-tile, m] = x @ W_in.T ----
            with (
                tc.tile_pool(name="w_in", bufs=1) as wpool,
                tc.tile_pool(name="xt", bufs=6) as xtpool,
            ):
                w_inT_sb = wpool.tile([128, KT * E], MM_DT, tag="w_inT_sb")

                def load_xt(i):
                    xt_t = xtpool.tile([128, KT * 128], MM_DT, tag="xt")
                    nc.sync.dma_start(
                        xt_t[:].rearrange("p (kt l) -> p kt l", kt=KT),
                        xT[:, i * 128 : (i + 1) * 128].rearrange(
                            "(kt p) l -> p kt l", p=128
                        ),
                    )
                    return xt_t

                # DMA issue order drives queue service order: the first
                # matmul needs only xt_0 + W_in[kt=0][:, :512].
                nc.sync.dma_start(
                    w_inT_sb[:, 0:512], w_inT[0:128, 0:512]
                )
                xt_first = load_xt(0)
                nc.sync.dma_start(
                    w_inT_sb[:, 512:E], w_inT[0:128, 512:E]
                )
                for kt in range(1, KT):
                    nc.sync.dma_start(
                        w_inT_sb[:, kt * E : (kt + 1) * E],
                        w_inT[kt * 128 : (kt + 1) * 128, :],
                    )

                for i in range(NT):
                    xt_t = xt_first if i == 0 else load_xt(i)
                    # interleave phase-2 table DMAs into the xt stream: early
                    # enough to be resident when phase 2 starts, late enough
                    # not to delay the phase-1 pipeline fill.
                    pv = ps_big.tile([128, E], DT, tag="pp")
                    # interior tiles skip v[:, 384:640]: heads 3/4 ('first'/
                    # 'last') only consume v rows {0:16, 2032:2048}, handled
                    # as a rank-1 update, so those columns are dead there.
                    if i in (0, NT - 1):
                        chunks = ((0, 512), (512, 512))
                    else:
                        chunks = ((0, 384), (640, 384))
                    for kt in range(KT):
                        lhsT = xt_t[:, kt * 128 : (kt + 1) * 128]
                        for m0, n in chunks:
                            nc.tensor.matmul(
                                pv[:, m0 : m0 + n],
                                lhsT,
                                w_inT_sb[:, kt * E + m0 : kt * E + m0 + n],
                                start=(kt == 0),
                                stop=(kt == KT - 1),
                            )
                    nc.vector.tensor_copy(v_sb[:, i * E : (i + 1) * E], pv[:])

            # phase-2 tables: issued after phase-1 DMAs so they don't delay
            # it; a_all first (attention consumes it before w_outT is needed)
            nc.sync.dma_start(a_sb[:], a_all[:])
            nc.sync.dma_start(wfl_sb[:], wfl[:])
            nc.sync.dma_start(ones_sb[:], ones[:])
            for kt in range(KT):
                nc.sync.dma_start(
                    w_outT_sb[:, kt * E : (kt + 1) * E],
                    w_outT[kt * 128 : (kt + 1) * 128, :],
                )

            if phases < 2:
                for i in range(NT):
                    ot = outpool.tile([128, E], DT, tag="out")
                    nc.scalar.copy(ot[:], v_sb[:, i * E : (i + 1) * E].bitcast(DT))
                    nc.sync.dma_start(out[i * 128 : (i + 1) * 128, :], ot[:])

            if phases >= 2:
                # ---- phase 2: per q-slab: attention, then output proj ----
                with tc.tile_pool(name="attp", bufs=2) as attpool:
                    for s in range(NSLAB):
                        att_sb = attpool.tile([128, NBH * 1024], MM_DT, tag="att")
                        for bi, h in enumerate(BANDED_HEADS):
                            t = HEAD_TYPE[h]
                            # collect this head's matmul pieces inside slab s
                            mms = []  # (col_in_slab, i, ci, bank)
                            for i in range(NT):
                                for start_rel, ci in _pieces(i):
                                    w0 = 128 * i + start_rel
                                    if not (1024 * s <= w0 < 1024 * (s + 1)):
                                        continue
                                    mms.append((w0 - 1024 * s, i, ci, w0 // 512))
                            last_of_bank = {}
                            for n_, mm in enumerate(mms):
                                last_of_bank[mm[3]] = n_
                            patt = ps_att.tile([128, 1024], DT, tag="patt")
                            started = set()
                            for n_, (col, i, ci, bank) in enumerate(mms):
                                first = bank not in started
                                started.add(bank)
                                nc.tensor.matmul(
                                    patt[:, col : col + 256],
                                    v_sb[:, i * E + h * 128 : i * E + (h + 1) * 128],
                                    ablk(t, ci),
                                    start=first,
                                    stop=(last_of_bank[bank] == n_),
                                )
                            nc.scalar.copy(
                                att_sb[:, bi * 1024 : (bi + 1) * 1024], patt[:]
                            )

                        if s == 0:
                            # ---- u vectors for 'first' (head 3) / 'last' (head 4) ----
                            pu = ps_big.tile([128, 2], DT, tag="pp")
                            nc.tensor.matmul(
                                pu[:, 0:1],
                                v_sb[:, 0 * E + 3 * 128 : 0 * E + 4 * 128].bitcast(DT),
                                wfl_sb[:, 0:1],
                                start=True,
                                stop=True,
                            )
                            nc.tensor.matmul(
                                pu[:, 1:2],
                                v_sb[:, 15 * E + 4 * 128 : 15 * E + 5 * 128].bitcast(DT),
                                wfl_sb[:, 1:2],
                                start=True,
                                stop=True,
                            )
                            u34_sb = cpool.tile([128, 2], MM_DT, tag="u34_sb")
                            nc.scalar.copy(u34_sb[:], pu[:])

                            # r34[e] = u3 @ W_outT[384:512, :] + u4 @ W_outT[512:640, :]
                            pr = ps_big.tile([1, E], DT, tag="pp")
                            for ec in range(2):
                                for hi, h in enumerate((3, 4)):
                                    nc.tensor.matmul(
                                        pr[:, ec * 512 : (ec + 1) * 512],
                                        u34_sb[:, hi : hi + 1],
                                        w_outT_sb[
                                            :, h * E + ec * 512 : h * E + ec * 512 + 512
                                        ],
                                        start=(hi == 0),
                                        stop=(hi == 1),
                                    )
                            r34_sb = cpool.tile([1, E], MM_DT, tag="r34_sb")
                            nc.scalar.copy(r34_sb[:], pr[:])

                            # broadcast r34 to all 128 partitions once (K=1 matmul),
                            # so the per-tile rank-1 update becomes a DVE add fused
                            # into the PSUM->SBUF out copy.
                            prb = ps_att.tile([128, 1024], DT, tag="patt")
                            for ec in range(2):
                                nc.tensor.matmul(
                                    prb[:, ec * 512 : (ec + 1) * 512],
                                    ones_sb[:],
                                    r34_sb[:, ec * 512 : (ec + 1) * 512],
                                    start=True,
                                    stop=True,
                                )
                            r34_full = cpool.tile([128, E], DT, tag="r34_full")
                            nc.scalar.copy(r34_full[:], prb[:])

                        for jj in range(8):  # q-tiles within slab
                            j = s * 8 + jj
                            po = ps_big.tile([128, E], DT, tag="pp")
                            for ec in range(2):
                                for bi, h in enumerate(BANDED_HEADS):
                                    nc.tensor.matmul(
                                        po[:, ec * 512 : (ec + 1) * 512],
                                        att_sb[
                                            :,
                                            bi * 1024
                                            + jj * 128 : bi * 1024
                                            + (jj + 1) * 128,
                                        ],
                                        w_outT_sb[
                                            :,
                                            h * E + ec * 512 : h * E + ec * 512 + 512,
                                        ],
                                        start=(bi == 0),
                                        stop=(bi == NBH - 1),
                                    )
                            out_t = outpool.tile([128, E], DT, tag="out")
                            nc.vector.tensor_add(out_t[:], po[:], r34_full[:])
                            nc.sync.dma_start(
                                out[j * 128 : (j + 1) * 128, :], out_t[:]
                            )

    nc.compile()
    return nc


class _Runner:
    """Builds the Bass program once and caches a jitted shard_map executable
    (one batch element per NeuronCore). Mirrors bass2jax.run_bass_via_pjrt
    but keeps the compiled callable + replicated weight arrays resident."""

    IN_ORDER = ["xT", "w_inT", "w_outT", "a_all", "wfl", "ones"]

    def __init__(self):
        import jax
        from jax.sharding import Mesh, PartitionSpec
        from jax.experimental.shard_map import shard_map

        self.jax = jax
        _b2j.install_neuronx_cc_hook()
        nc = _build_program()
        self.nc = nc
        self.a_all_np, self.wfl_np = _host_tables()

        partition_name = (
            nc.partition_id_tensor.name if nc.partition_id_tensor else None
        )
        in_names = []
        out_names = []
        out_avals = []
        for alloc in nc.m.functions[0].allocations:
            if not isinstance(alloc, mybir.MemoryLocationSet):
                continue
            name = alloc.memorylocations[0].name
            if alloc.kind == "ExternalInput":
                if name != partition_name:
                    in_names.append(name)
            elif alloc.kind == "ExternalOutput":
                out_names.append(name)
                out_avals.append(
                    jax.core.ShapedArray(
                        tuple(alloc.tensor_shape), mybir.dt.np(alloc.dtype)
                    )
                )
        assert sorted(in_names) == sorted(self.IN_ORDER), in_names
        self.in_names = in_names
        self.out_names = out_names
        self.out_avals = out_avals
        n_params = len(in_names)
        n_outs = len(out_names)
        all_names = tuple(in_names) + tuple(out_names)
        if partition_name is not None:
            all_names = all_names + (partition_name,)

        def _body(*args):
            operands = list(args)
            if partition_name is not None:
                operands.append(_b2j.partition_id_tensor())
            outs = _b2j._bass_exec_p.bind(
                *operands,
                out_avals=tuple(out_avals),
                in_names=all_names,
                out_names=tuple(out_names),
                lowering_input_output_aliases=(),
                sim_require_finite=True,
                sim_require_nnan=True,
                nc=nc,
            )
            return tuple(outs)

        devices = jax.devices()[:B]
        assert len(devices) == B
        self.mesh = Mesh(np.asarray(devices), ("core",))
        in_specs = (PartitionSpec("core"),) * (n_params + n_outs)
        out_specs = (PartitionSpec("core"),) * n_outs
        self.sharded = jax.jit(
            shard_map(
                _body,
                mesh=self.mesh,
                in_specs=in_specs,
                out_specs=out_specs,
                check_rep=False,
            ),
            donate_argnums=tuple(range(n_params, n_params + n_outs)),
            keep_unused=True,
        )

    def _concat_static(self, w_inT, w_outT):
        jax = self.jax
        statics = {
            "w_inT": w_inT,
            "w_outT": w_outT,
            "a_all": self.a_all_np,
            "wfl": self.wfl_np,
            "ones": np.ones((1, 128), dtype=np.float32),
        }
        out = {}
        for name, arr in statics.items():
            big = np.concatenate([arr] * B, axis=0)
            out[name] = jax.device_put(big)
        return out

    def run_device(self, dev_args):
        jnp = self.jax.numpy
        zeros = [
            jnp.zeros((B * av.shape[0], *av.shape[1:]), av.dtype)
            for av in self.out_avals
        ]
        return self.sharded(*dev_args, *zeros)

    def prepare_inputs(self, x, W_in, W_out):
        xT_np = np.ascontiguousarray(x.transpose(0, 2, 1)).reshape(B * E, L)
        w_inT_np = np.ascontiguousarray(W_in.T)
        w_outT_np = np.ascontiguousarray(W_out.T)
        dev = self._concat_static(w_inT_np, w_outT_np)
        dev["xT"] = self.jax.device_put(xT_np)
        return [dev[name] for name in self.in_names]

    def __call__(self, x, W_in, W_out):
        args = self.prepare_inputs(x, W_in, W_out)
        outs = self.run_device(args)
        out = np.asarray(outs[self.out_names.index("out")])
        return out.reshape(B, L, E)


_CACHE = {}


def _get_runner() -> _Runner:
    if "runner" not in _CACHE:
        _CACHE["runner"] = _Runner()
    return _CACHE["runner"]


def kernel(x, W_in, W_out):
    x = np.ascontiguousarray(np.asarray(x, dtype=np.float32))
    W_in = np.ascontiguousarray(np.asarray(W_in, dtype=np.float32))
    W_out = np.ascontiguousarray(np.asarray(W_out, dtype=np.float32))
    assert x.shape == (B, L, E)
    return _get_runner()(x, W_in, W_out)


if __name__ == "__main__":
    rng = np.random.default_rng(0)
    x = rng.standard_normal((B, L, E), dtype=np.float32)
    W_in = rng.standard_normal((E, E), dtype=np.float32) * 0.05
    W_out = rng.standard_normal((E, E), dtype=np.float32) * 0.05
    y = kernel(x, W_in, W_out)
    print("out", y.shape, y.dtype, np.abs(y).mean())



# revision 29
# speedup vs baseline: 1.4893x; 1.4893x over previous
"""Trainium2 Bass kernel for nn_NewAttention (analytic Gaussian sparse attention).

Math (per batch element b):
    v        = x[b] @ W_in.T                      # [L, E]
    per head h (P=128 cols of v):
        A_h  = softmax(-(j - c_h(i))^2 / 2)       # [L, L], analytic, banded
        att_h = A_h @ v_h                         # [L, P]
    out[b]   = concat_h(att_h) @ W_out.T          # [L, E]

Sharding: data-parallel over batch, one batch element per NeuronCore (8 cores).

Device strategy (per core):
  - 'first'/'last' heads (3/4) attend to a fixed key location for every query,
    so their output contribution is a single row vector r34[e] added to every
    output row. r34 only depends on 32 rows of x, so the HOST computes it
    exactly and ships it as a per-core [128, 8] bias table. Heads 3/4 then
    vanish from the device program entirely.
  - matmul1 (v = x @ W_in.T): fp8e4 DoubleRow with a dual-plane
    error-compensated split: x = xh + xl, 64*W = Wh + Wl (same scale for all
    planes), accumulate xh@Wh + xh@Wl + xl@Wh in one PSUM group; the dropped
    xl@Wl term and plane-residuals are ~0.2% — bf16-level accuracy at half
    the PE cost. The 1/64 descale rides the PSUM->SBUF copy for free.
  - attention: att^T_h = v_h.T @ A_h^T as banded bf16 matmuls: stationary =
    v 128x128 slices, moving = analytic A^T window blocks (host-precomputed
    exact softmax weights, truncated at |key-center| <= 4). All interior tiles
    share ONE shift-invariant [128, 136] window table; boundary tiles get
    exact renormalized tables. Windows split at PSUM-bank (512 col)
    boundaries and accumulate via per-element has_written bits. Attention
    column-groups are emitted inside the phase-1 tile loop as soon as their
    v tiles exist, so their PSUM->SBUF copies hide under phase-1 matmuls.
  - matmul2 computes out^T (feature-major): stationary = W_out^T slices,
    moving = att^T q-chunks, accumulated over the 6 banded heads — also in
    dual-plane fp8 DoubleRow (att planes are split on-device: hi = 8*att via
    one copy, lo = (8*att - hi) via one scalar_tensor_tensor). The r34 bias
    and the 1/(8*64) descale ride the PSUM->SBUF copy. Output leaves the
    device as bf16 out^T; the host transposes and upcasts.
  - PE p-state: dummy matmuls on a zeroed scratch tile run during the initial
    DMA fill so the clock ramp completes before real work arrives.
"""

import os
import sys
import numpy as np

for _p in ("/opt/trn_rl_repo",):
    if _p not in sys.path:
        sys.path.insert(0, _p)

import concourse.bass as bass
import concourse.bacc as bacc
import concourse.mybir as mybir
from concourse import tile
from concourse import bass2jax as _b2j

# ---------------- problem constants (hardcoded per contract) ----------------
B = 8
L = 2048
E = 1024
H = 8
P = 128
SIGMA = 1.0
DISP = 1
NT = L // 128           # 16 seq tiles
R = 4                   # Gaussian taps kept each side of the center
WIN = 128 + 2 * R       # 136: per-tile attention query window
DT = mybir.dt.float32
BF = mybir.dt.bfloat16
F8 = mybir.dt.float8e4

GEMM1_FP8 = os.environ.get("K_GEMM1_FP8", "1") == "1"
GEMM2_FP8 = os.environ.get("K_GEMM2_FP8", "1") == "1"
W_SCALE = 64.0          # fp8 plane scale for W_in / W_out
ATT_SCALE = 8.0         # fp8 plane scale for att^T

BANDED_HEADS = [0, 1, 2, 5, 6, 7]   # center,left,right,center,left,right
NBH = len(BANDED_HEADS)
BI_TYPE = [0, 1, 2, 0, 1, 2]        # 0=center,1=left,2=right
TYPE_DISP = [0, -DISP, +DISP]
NQ4 = L // 512                      # 4 attention PSUM column groups
# emit attention (q4 group, head pair) at the end of phase-1 tile iteration i
ATT_EMIT_AT = {
    5: (0, 0), 6: (0, 1), 7: (0, 2),
    9: (1, 0), 10: (1, 1), 11: (1, 2),
    13: (2, 0), 14: (2, 1), 15: (2, 2),
}


def _g(x):
    return np.exp(-(np.asarray(x, dtype=np.float64) ** 2) / (2.0 * SIGMA**2))


def _attn_tables():
    """[128, 7*136] float: interior | first(t=0..2) | last(t=0..2) A^T blocks.

    interior[k, c] = g(k + R - c)/Zinf  (shift-invariant, shared by all head
    types and tiles 1..14; the head displacement only moves the window).
    first/last blocks are exact full-softmax values at the sequence edges.
    """
    tab = np.zeros((128, 7 * WIN), dtype=np.float64)
    zinf = _g(np.arange(-64, 65)).sum()
    k = np.arange(128)
    c = np.arange(WIN)
    delta = k[:, None] + R - c[None, :]
    tab[:, 0:WIN] = np.where(np.abs(delta) <= R, _g(delta) / zinf, 0.0)

    j = np.arange(L, dtype=np.float64)
    for t in range(3):
        d = TYPE_DISP[t]
        # first block: tile 0, queries q in [0, 132 - d)
        w = 132 - d
        q = np.arange(w, dtype=np.float64)
        logits = _g(j[None, :] - (q[:, None] + d))          # [w, L]
        A = logits / logits.sum(axis=1, keepdims=True)
        tab[:, (1 + t) * WIN : (1 + t) * WIN + w] = A[:, 0:128].T
        # last block: tile 15, queries q in [1916 - d, 2048)
        w = 132 + d
        q = np.arange(L - w, L, dtype=np.float64)
        logits = _g(j[None, :] - (q[:, None] + d))
        A = logits / logits.sum(axis=1, keepdims=True)
        tab[:, (4 + t) * WIN : (4 + t) * WIN + w] = A[:, L - 128 : L].T
    return tab


def _attn_pieces():
    """pieces[t][q4] = ordered [(tile i, col within 512-psum, width, table col)]."""
    pieces = [[[] for _ in range(NQ4)] for _ in range(3)]
    for t in range(3):
        d = TYPE_DISP[t]
        for i in range(NT):
            if i == 0:
                w0, w, base = 0, 132 - d, (1 + t) * WIN
            elif i == NT - 1:
                w0, w, base = 128 * i - R - d, 132 + d, (4 + t) * WIN
            else:
                w0, w, base = 128 * i - R - d, WIN, 0
            p0 = w0
            while p0 < w0 + w:
                q4 = p0 // 512
                pend = min(w0 + w, (q4 + 1) * 512)
                pieces[t][q4].append((i, p0 - 512 * q4, pend - p0, base + p0 - w0))
                p0 = pend
    return pieces


ATT_PIECES = _attn_pieces()

# GEMM1 column chunks: banded heads only (0-2 -> [0,384), 5-7 -> [640,1024))
G1_CHUNKS = ((0, 384), (640, 384))


def _build_program(phases=3):
    nc = bacc.Bacc("TRN2", target_bir_lowering=False, debug=False, num_devices=B)

    # w_in ships only the 6 banded heads' 6144 output columns, m-half-major:
    # [mh][kc/kt][(i)][384] so each m-half is one contiguous early DMA.
    if GEMM1_FP8:
        xt8 = nc.dram_tensor("xt8", [L, 2 * E], F8, kind="ExternalInput")
        w_in_hi = nc.dram_tensor("w_in_hi", [128, 6144], F8, kind="ExternalInput")
        w_in_lo = nc.dram_tensor("w_in_lo", [128, 6144], F8, kind="ExternalInput")
    else:
        xt = nc.dram_tensor("xt", [L, E], BF, kind="ExternalInput")
        w_in = nc.dram_tensor("w_in", [128, 6144], BF, kind="ExternalInput")
    if GEMM2_FP8:
        w_out_hi = nc.dram_tensor("w_out_hi", [128, NBH * E], F8, kind="ExternalInput")
        w_out_lo = nc.dram_tensor("w_out_lo", [128, NBH * E], F8, kind="ExternalInput")
    else:
        w_out = nc.dram_tensor("w_out", [128, 8 * E], BF, kind="ExternalInput")
    a_tab = nc.dram_tensor("a_tab", [128, 7 * WIN], BF, kind="ExternalInput")
    r34t = nc.dram_tensor("r34t", [128, 8], DT, kind="ExternalInput")
    outT = nc.dram_tensor("outT", [E, L], BF, kind="ExternalOutput")

    with tile.TileContext(nc) as tc:
        with (
            tc.tile_pool(name="const", bufs=1) as cpool,
            tc.tile_pool(name="vbuf", bufs=1) as vpool,
            tc.tile_pool(name="outp", bufs=4) as outpool,
            tc.tile_pool(name="ps_pv", bufs=2, space="PSUM") as ps_pv,
            tc.tile_pool(name="ps_att", bufs=4, space="PSUM") as ps_att,
        ):
            if GEMM2_FP8:
                wo_hi_sb = cpool.tile([128, NBH * E], F8, tag="wo_hi_sb")
                wo_lo_sb = cpool.tile([128, NBH * E], F8, tag="wo_lo_sb")
                att_hi = cpool.tile([128, 2 * NBH * E], F8, tag="att_hi")
                att_lo = cpool.tile([128, 2 * NBH * E], F8, tag="att_lo")
            else:
                w_out_sb = cpool.tile([128, 8 * E], BF, tag="w_out_sb")
                att_sb = cpool.tile([128, 2 * NBH * E], BF, tag="att_sb")
            a_sb = cpool.tile([128, 7 * WIN], BF, tag="a_sb")
            r34_sb = cpool.tile([128, 8], DT, tag="r34_sb")
            scratch = cpool.tile([128, 512], BF, tag="scratch")
            v_sb = vpool.tile([128, NT * E], BF, tag="v_sb")

            # ---- PE warmup: ramp the p-state during the DMA fill ----
            nc.vector.memset(scratch[:, 0:128], 0.0)
            nc.vector.memset(scratch[:, 128:512], 0.0)
            pw = ps_att.tile([128, 512], DT, tag="patt")
            for _ in range(3):
                nc.tensor.matmul(
                    pw[:, 0:128], scratch[:, 0:128], scratch[:, 0:128],
                    start=True, stop=True,
                )
            for _ in range(6):
                nc.tensor.matmul(
                    pw[:], scratch[:, 0:128], scratch[:, 0:512],
                    start=True, stop=True,
                )

            # attention helpers -------------------------------------------
            copy_rr = [0]

            def emit_att_group(q4, pair=None):
                heads = list(enumerate(BANDED_HEADS))
                if pair is not None:
                    heads = heads[2 * pair : 2 * pair + 2]
                for bi, h in heads:
                    ms = ATT_PIECES[BI_TYPE[bi]][q4]
                    if q4 == 3 and bi % 2 == 1:
                        # borrow the (now idle) pv pool so the trailing q4=3
                        # copies don't starve GEMM2's PSUM slot rotation
                        patt_t = ps_pv.tile([128, E], DT, tag="pv")
                        patt = patt_t[:, 0:512]
                    else:
                        patt_t = ps_att.tile([128, 512], DT, tag="patt")
                        patt = patt_t[:]
                    for n_, (i, col, wd, tcol) in enumerate(ms):
                        nc.tensor.matmul(
                            patt[:, col : col + wd],
                            v_sb[:, i * E + h * 128 : i * E + (h + 1) * 128],
                            a_sb[:, tcol : tcol + wd],
                            start=(n_ == 0),
                            stop=(n_ == len(ms) - 1),
                        )
                    s, qq = q4 // 2, q4 % 2
                    if GEMM2_FP8:
                        c, j = bi // 2, bi % 2
                        base = ((s * 3 + c) * 2 + j) * E + qq * 512
                        hi_dst = att_hi[:, base : base + 512]
                        lo_dst = att_lo[:, base : base + 512]
                        nc.scalar.activation(
                            hi_dst, patt,
                            mybir.ActivationFunctionType.Copy,
                            scale=ATT_SCALE,
                        )
                        nc.vector.scalar_tensor_tensor(
                            lo_dst, patt, ATT_SCALE, hi_dst,
                            mybir.AluOpType.mult, mybir.AluOpType.subtract,
                        )
                    else:
                        dst = att_sb[:, (s * NBH + bi) * E + qq * 512 :][:, :512]
                        if copy_rr[0] % 2 == 0:
                            nc.scalar.copy(dst, patt)
                        else:
                            nc.vector.tensor_copy(dst, patt)
                    copy_rr[0] += 1

            # ---- phase 1: v = x @ W_in.T (banded-head columns only) ----
            with (
                tc.tile_pool(name="w_in_p", bufs=1) as wpool,
                tc.tile_pool(name="xt_p", bufs=7) as xtpool,
            ):
                if GEMM1_FP8:
                    w_hi_sb = wpool.tile([128, 6144], F8, tag="w_hi_sb")
                    w_lo_sb = wpool.tile([128, 6144], F8, tag="w_lo_sb")

                    def load_xt(i):
                        t8 = xtpool.tile([128, 2 * E], F8, tag="xt")
                        nc.sync.dma_start(t8[:], xt8[i * 128 : (i + 1) * 128, :])
                        return t8[:, 0:E], t8[:, E : 2 * E]

                    def load_w(mh):
                        nc.sync.dma_start(
                            w_hi_sb[:, mh * 3072 : (mh + 1) * 3072],
                            w_in_hi[:, mh * 3072 : (mh + 1) * 3072],
                        )
                        nc.sync.dma_start(
                            w_lo_sb[:, mh * 3072 : (mh + 1) * 3072],
                            w_in_lo[:, mh * 3072 : (mh + 1) * 3072],
                        )
                else:
                    w_in_sb = wpool.tile([128, 6144], BF, tag="w_in_sb")

                    def load_xt(i):
                        t_ = xtpool.tile([128, E], BF, tag="xt")
                        nc.sync.dma_start(t_[:], xt[i * 128 : (i + 1) * 128, :])
                        return t_

                    def load_w(mh):
                        nc.sync.dma_start(
                            w_in_sb[:, mh * 3072 : (mh + 1) * 3072],
                            w_in[:, mh * 3072 : (mh + 1) * 3072],
                        )

                # stream in consumption order: w m-half 0, xt0/1, m-half 1
                load_w(0)
                xts = [load_xt(0)]
                xts.append(load_xt(1))
                load_w(1)
                xts.append(load_xt(2))
                xts.append(load_xt(3))

                pvs = {}
                vsc = (1.0 / W_SCALE) if GEMM1_FP8 else 1.0

                def g1_chunk(i, mh):
                    m0, n = G1_CHUNKS[mh]
                    pv = pvs[i]
                    if GEMM1_FP8:
                        th, tl = xts[i]
                        for kc in range(4):
                            sh = th[:, kc * 256 : (kc + 1) * 256].rearrange(
                                "p (i l) -> p i l", i=2
                            )
                            sl = tl[:, kc * 256 : (kc + 1) * 256].rearrange(
                                "p (i l) -> p i l", i=2
                            )
                            wh = w_hi_sb[
                                :, (mh * 4 + kc) * 768 : (mh * 4 + kc + 1) * 768
                            ].rearrange("p (i m) -> p i m", i=2)
                            wl = w_lo_sb[
                                :, (mh * 4 + kc) * 768 : (mh * 4 + kc + 1) * 768
                            ].rearrange("p (i m) -> p i m", i=2)
                            for term, (s_, m_) in enumerate(
                                ((sh, wh), (sh, wl), (sl, wh))
                            ):
                                nc.tensor.matmul(
                                    pv[:, m0 : m0 + n],
                                    s_,
                                    m_,
                                    start=(kc == 0 and term == 0),
                                    stop=(kc == 3 and term == 2),
                                    perf_mode=mybir.MatmulPerfMode.DoubleRow,
                                )
                    else:
                        xt_t = xts[i]
                        for kt in range(8):
                            nc.tensor.matmul(
                                pv[:, m0 : m0 + n],
                                xt_t[:, kt * 128 : (kt + 1) * 128],
                                w_in_sb[
                                    :, (mh * 8 + kt) * 384 : (mh * 8 + kt + 1) * 384
                                ],
                                start=(kt == 0),
                                stop=(kt == 7),
                            )

                def g1_copy(i, mh):
                    m0, n = G1_CHUNKS[mh]
                    pv = pvs[i]
                    dst = v_sb[:, i * E + m0 : i * E + m0 + n]
                    if (i + mh) % 2 == 0:
                        nc.scalar.activation(
                            dst, pv[:, m0 : m0 + n],
                            mybir.ActivationFunctionType.Copy, scale=vsc,
                        )
                    else:
                        if GEMM1_FP8:
                            nc.vector.tensor_scalar_mul(dst, pv[:, m0 : m0 + n], vsc)
                        else:
                            nc.vector.tensor_copy(dst, pv[:, m0 : m0 + n])

                # tiles 0/1 interleave m-halves so PE work tracks DMA arrival
                pv0 = ps_pv.tile([128, E], DT, tag="pv")
                pv1 = ps_pv.tile([128, E], DT, tag="pv")
                pvs[0], pvs[1] = pv0, pv1
                for i_, mh_ in ((0, 0), (1, 0), (0, 1), (1, 1)):
                    g1_chunk(i_, mh_)
                    g1_copy(i_, mh_)

                xts.append(load_xt(4))
                xts.append(load_xt(5))
                nc.sync.dma_start(a_sb[:], a_tab[:])
                nc.sync.dma_start(r34_sb[:], r34t[:])

                for i in range(2, NT):
                    if i + 4 < NT:
                        xts.append(load_xt(i + 4))
                    if GEMM2_FP8:
                        if 4 <= i < 7:
                            c = i - 4
                            nc.sync.dma_start(
                                wo_hi_sb[:, c * 2048 : (c + 1) * 2048],
                                w_out_hi[:, c * 2048 : (c + 1) * 2048],
                            )
                            nc.sync.dma_start(
                                wo_lo_sb[:, c * 2048 : (c + 1) * 2048],
                                w_out_lo[:, c * 2048 : (c + 1) * 2048],
                            )
                    else:
                        if 4 <= i < 12:
                            c = i - 4
                            nc.sync.dma_start(
                                w_out_sb[:, c * 1024 : (c + 1) * 1024],
                                w_out[:, c * 1024 : (c + 1) * 1024],
                            )
                    pv_i = ps_pv.tile([128, E], DT, tag="pv")
                    pvs[i] = pv_i
                    for mh_ in (0, 1):
                        g1_chunk(i, mh_)
                    for mh_ in (0, 1):
                        g1_copy(i, mh_)
                    if i in ATT_EMIT_AT:
                        q4_, pair_ = ATT_EMIT_AT[i]
                        emit_att_group(q4_, pair_)

            emit_att_group(3)

            # ---- phase 3: out^T = W_out @ att^T + r34 bias ----
            osc = 1.0 / (ATT_SCALE * W_SCALE)
            nout = 0
            for qc in range(4):
                s, qq = qc // 2, qc % 2
                for et in range(8):
                    if (qc * 8 + et) % 2 == 1:
                        po_t = ps_att.tile([128, 512], DT, tag="patt")
                        po = po_t[:]
                    else:
                        po_t = ps_pv.tile([128, E], DT, tag="pv")
                        po = po_t[:, 0:512]
                    # final chunk splits so the very last output DMA is tiny;
                    # sub-chunk 2 gets its own PSUM bank so its matmuls don't
                    # serialize behind sub-chunk 1's PSUM read (bank tracker)
                    last = qc == 3 and et == 7
                    subs = ((0, 384), (384, 128)) if last else ((0, 512),)
                    for si, (s0, sn) in enumerate(subs):
                        if si == 1:
                            po_t = ps_att.tile([128, 512], DT, tag="patt")
                            po = po_t[:]
                        p0 = 0 if si == 1 else s0
                        if GEMM2_FP8:
                            nmm = 0
                            for c in range(3):
                                rh = att_hi[
                                    :, (s * 3 + c) * 2048 :][:, :2048].rearrange(
                                    "p (j q) -> p j q", j=2
                                )[:, :, qq * 512 + s0 : qq * 512 + s0 + sn]
                                rl = att_lo[
                                    :, (s * 3 + c) * 2048 :][:, :2048].rearrange(
                                    "p (j q) -> p j q", j=2
                                )[:, :, qq * 512 + s0 : qq * 512 + s0 + sn]
                                lh = wo_hi_sb[
                                    :, c * 2048 : (c + 1) * 2048
                                ].rearrange("p (j e) -> p j e", j=2)[
                                    :, :, et * 128 : (et + 1) * 128
                                ]
                                ll = wo_lo_sb[
                                    :, c * 2048 : (c + 1) * 2048
                                ].rearrange("p (j e) -> p j e", j=2)[
                                    :, :, et * 128 : (et + 1) * 128
                                ]
                                for s_, m_ in ((lh, rh), (ll, rh), (lh, rl)):
                                    nc.tensor.matmul(
                                        po[:, p0 : p0 + sn], s_, m_,
                                        start=(nmm == 0),
                                        stop=(nmm == 8),
                                        perf_mode=mybir.MatmulPerfMode.DoubleRow,
                                    )
                                    nmm += 1
                        else:
                            for bi, h in enumerate(BANDED_HEADS):
                                nc.tensor.matmul(
                                    po[:, p0 : p0 + sn],
                                    w_out_sb[
                                        :, h * E + et * 128 : h * E + (et + 1) * 128
                                    ],
                                    att_sb[
                                        :, (s * NBH + bi) * E + qq * 512 + s0 :
                                    ][:, :sn],
                                    start=(bi == 0),
                                    stop=(bi == NBH - 1),
                                )
                        ot = outpool.tile([128, 512], BF, tag="ot")
                        bias = r34_sb[:, et : et + 1]
                        if GEMM2_FP8:
                            if nout % 2 == 0:
                                nc.scalar.activation(
                                    ot[:, 0:sn], po[:, p0 : p0 + sn],
                                    mybir.ActivationFunctionType.Identity,
                                    bias=bias, scale=osc,
                                )
                            else:
                                nc.vector.tensor_scalar(
                                    ot[:, 0:sn], po[:, p0 : p0 + sn], osc, bias,
                                    mybir.AluOpType.mult, mybir.AluOpType.add,
                                )
                        else:
                            if nout % 2 == 0:
                                nc.scalar.add(ot[:, 0:sn], po[:, p0 : p0 + sn], bias)
                            else:
                                nc.vector.tensor_scalar_add(
                                    ot[:, 0:sn], po[:, p0 : p0 + sn], bias
                                )
                        nout += 1
                        nc.sync.dma_start(
                            outT[
                                et * 128 : (et + 1) * 128,
                                qc * 512 + s0 : qc * 512 + s0 + sn,
                            ],
                            ot[:, 0:sn],
                        )

    nc.compile()
    return nc


# ------------------------- host-side preparation ---------------------------

_NPBF = mybir.dt.np(BF)
_NPF8 = mybir.dt.np(F8)
# banded-head output columns of W_in^T, m-half-major (0:384 then 640:1024)
_MCOLS = np.concatenate([np.arange(0, 384), np.arange(640, 1024)])


def _host_wf_wl():
    """Exact 'first'/'last' head weight vectors over their 16-key support."""
    j = np.arange(L, dtype=np.float64)
    zf = _g(j - 0.0).sum()
    zl = _g(j - (L - 1.0)).sum()
    wf = _g(np.arange(16)) / zf
    wl = _g(np.arange(L - 16, L) - (L - 1.0)) / zl
    return wf, wl


def _host_r34(x, W_in, W_out):
    """[B, 128, 8] fp32: per-core output bias rows from the 'first'/'last'
    heads, computed exactly on the host (r34t[p, et] = r34[et*128 + p])."""
    wf, wl = _host_wf_wl()
    x64 = x.astype(np.float64)
    s3 = np.einsum("k,bke->be", wf, x64[:, 0:16, :])        # [B, E]
    s4 = np.einsum("k,bke->be", wl, x64[:, L - 16 : L, :])
    W_in64 = W_in.astype(np.float64)
    W_out64 = W_out.astype(np.float64)
    u3 = s3 @ W_in64.T[:, 384:512]                          # [B, 128]
    u4 = s4 @ W_in64.T[:, 512:640]
    r34 = u3 @ W_out64.T[384:512, :] + u4 @ W_out64.T[512:640, :]  # [B, E]
    return np.ascontiguousarray(
        r34.reshape(B, 8, 128).transpose(0, 2, 1)
    ).astype(np.float32)


def _pack_xt_bf16(x):
    # xt[b, i*128 + p, kt*128 + l] = x[b, i*128 + l, kt*128 + p]
    t = x.reshape(B, NT, 128, 8, 128).transpose(0, 1, 4, 3, 2)
    return np.ascontiguousarray(t).reshape(B * L, E).astype(_NPBF)


def _pack_xt_fp8(xq):
    # xt[b, i*128 + p, kc*256 + ipl*128 + l] = xq[b, i*128 + l, kc*256 + ipl*128 + p]
    t = xq.reshape(B, NT, 128, 4, 2, 128).transpose(0, 1, 5, 3, 4, 2)
    return np.ascontiguousarray(t).reshape(B * L, E)


def _pack_w_bf16(Wt):
    # w[p, (mh*8 + kt)*384 + m] = W.T[kt*128 + p, mcol(mh, m)]
    t = Wt.reshape(8, 128, E)[:, :, _MCOLS]          # [kt, p, mh*384+m]
    t = t.reshape(8, 128, 2, 384).transpose(1, 2, 0, 3)
    return np.ascontiguousarray(t).reshape(128, 6144).astype(_NPBF)


def _pack_w_fp8(Wq):
    # w[p, ((mh*4 + kc)*2 + ipl)*384 + m] = Wq[kc*256 + ipl*128 + p, mcol(mh, m)]
    t = Wq.reshape(4, 2, 128, E)[:, :, :, _MCOLS]    # [kc, ipl, p, mh*384+m]
    t = t.reshape(4, 2, 128, 2, 384).transpose(2, 3, 0, 1, 4)
    return np.ascontiguousarray(t).reshape(128, 6144)


def _pack_wo_fp8(Wq):
    # Wq: [NBH*128, E] rows = banded-head-major features (bi, p).
    # wo[p, c*2048 + j*1024 + e] = Wq[(c*2 + j)*128 + p, e]
    t = Wq.reshape(3, 2, 128, E).transpose(2, 0, 1, 3)
    return np.ascontiguousarray(t).reshape(128, NBH * E)


def _split_f8(a):
    hi = a.astype(_NPF8)
    lo = (a - hi.astype(np.float32)).astype(_NPF8)
    return hi, lo


class _Runner:
    """Builds the Bass program once and caches a jitted shard_map executable
    (one batch element per NeuronCore)."""

    def __init__(self):
        import jax
        from jax.sharding import Mesh, PartitionSpec
        from jax.experimental.shard_map import shard_map

        self.jax = jax
        _b2j.install_neuronx_cc_hook()
        nc = _build_program()
        self.nc = nc
        self.a_tab_np = _attn_tables().astype(_NPBF)

        partition_name = (
            nc.partition_id_tensor.name if nc.partition_id_tensor else None
        )
        in_names = []
        out_names = []
        out_avals = []
        for alloc in nc.m.functions[0].allocations:
            if not isinstance(alloc, mybir.MemoryLocationSet):
                continue
            name = alloc.memorylocations[0].name
            if alloc.kind == "ExternalInput":
                if name != partition_name:
                    in_names.append(name)
            elif alloc.kind == "ExternalOutput":
                out_names.append(name)
                out_avals.append(
                    jax.core.ShapedArray(
                        tuple(alloc.tensor_shape), mybir.dt.np(alloc.dtype)
                    )
                )
        self.in_names = in_names
        self.out_names = out_names
        self.out_avals = out_avals
        n_params = len(in_names)
        n_outs = len(out_names)
        all_names = tuple(in_names) + tuple(out_names)
        if partition_name is not None:
            all_names = all_names + (partition_name,)

        def _body(*args):
            operands = list(args)
            if partition_name is not None:
                operands.append(_b2j.partition_id_tensor())
            outs = _b2j._bass_exec_p.bind(
                *operands,
                out_avals=tuple(out_avals),
                in_names=all_names,
                out_names=tuple(out_names),
                lowering_input_output_aliases=(),
                sim_require_finite=True,
                sim_require_nnan=True,
                nc=nc,
            )
            return tuple(outs)

        devices = jax.devices()[:B]
        assert len(devices) == B
        self.mesh = Mesh(np.asarray(devices), ("core",))
        in_specs = (PartitionSpec("core"),) * (n_params + n_outs)
        out_specs = (PartitionSpec("core"),) * n_outs
        self.sharded = jax.jit(
            shard_map(
                _body,
                mesh=self.mesh,
                in_specs=in_specs,
                out_specs=out_specs,
                check_rep=False,
            ),
            donate_argnums=tuple(range(n_params, n_params + n_outs)),
            keep_unused=True,
        )

    def run_device(self, dev_args):
        jnp = self.jax.numpy
        zeros = [
            jnp.zeros((B * av.shape[0], *av.shape[1:]), av.dtype)
            for av in self.out_avals
        ]
        return self.sharded(*dev_args, *zeros)

    def prepare_inputs(self, x, W_in, W_out):
        jax = self.jax
        dev = {}
        if GEMM1_FP8:
            xh, xl = _split_f8(x)
            dev["xt8"] = np.concatenate(
                [_pack_xt_fp8(xh), _pack_xt_fp8(xl)], axis=1
            )
            Wt = np.ascontiguousarray(W_in.T) * np.float32(W_SCALE)
            Wh, Wl = _split_f8(Wt)
            dev["w_in_hi"] = np.concatenate([_pack_w_fp8(Wh)] * B, axis=0)
            dev["w_in_lo"] = np.concatenate([_pack_w_fp8(Wl)] * B, axis=0)
        else:
            dev["xt"] = _pack_xt_bf16(x)
            w_in_b = _pack_w_bf16(np.ascontiguousarray(W_in.T))
            dev["w_in"] = np.concatenate([w_in_b] * B, axis=0)
        if GEMM2_FP8:
            rows = np.concatenate(
                [np.arange(h * 128, (h + 1) * 128) for h in BANDED_HEADS]
            )
            Wq = np.ascontiguousarray(W_out.T[rows, :]) * np.float32(W_SCALE)
            Wh, Wl = _split_f8(Wq)
            dev["w_out_hi"] = np.concatenate([_pack_wo_fp8(Wh)] * B, axis=0)
            dev["w_out_lo"] = np.concatenate([_pack_wo_fp8(Wl)] * B, axis=0)
        else:
            w_out_b = _pack_w_bf16(np.ascontiguousarray(W_out.T))
            dev["w_out"] = np.concatenate([w_out_b] * B, axis=0)
        dev["a_tab"] = np.concatenate([self.a_tab_np] * B, axis=0)
        dev["r34t"] = _host_r34(x, W_in, W_out).reshape(B * 128, 8)
        return [jax.device_put(dev[name]) for name in self.in_names]

    def __call__(self, x, W_in, W_out):
        args = self.prepare_inputs(x, W_in, W_out)
        outs = self.run_device(args)
        outT = np.asarray(outs[self.out_names.index("outT")])
        # outT: [B*E, L] bf16 -> [B, L, E] fp32
        return np.ascontiguousarray(
            outT.reshape(B, E, L).transpose(0, 2, 1)
        ).astype(np.float32)


_CACHE = {}


def _get_runner() -> _Runner:
    if "runner" not in _CACHE:
        _CACHE["runner"] = _Runner()
    return _CACHE["runner"]


def kernel(x, W_in, W_out):
    x = np.ascontiguousarray(np.asarray(x, dtype=np.float32))
    W_in = np.ascontiguousarray(np.asarray(W_in, dtype=np.float32))
    W_out = np.ascontiguousarray(np.asarray(W_out, dtype=np.float32))
    assert x.shape == (B, L, E)
    return _get_runner()(x, W_in, W_out)


if __name__ == "__main__":
    rng = np.random.default_rng(0)
    x = rng.standard_normal((B, L, E), dtype=np.float32)
    W_in = rng.standard_normal((E, E), dtype=np.float32) * 0.05
    W_out = rng.standard_normal((E, E), dtype=np.float32) * 0.05
    y = kernel(x, W_in, W_out)
    print("out", y.shape, y.dtype, np.abs(y).mean())


# revision 35
# speedup vs baseline: 1.5184x; 1.0195x over previous
"""Trainium2 Bass kernel for nn_NewAttention (analytic Gaussian sparse attention).

Math (per batch element b):
    v        = x[b] @ W_in.T                      # [L, E]
    per head h (P=128 cols of v):
        A_h  = softmax(-(j - c_h(i))^2 / 2)       # [L, L], analytic, banded
        att_h = A_h @ v_h                         # [L, P]
    out[b]   = concat_h(att_h) @ W_out.T          # [L, E]

Sharding: data-parallel over batch, one batch element per NeuronCore (8 cores).

Device strategy (per core):
  - 'first'/'last' heads (3/4) attend to a fixed key location for every query,
    so their output contribution is a single row vector r34[e] added to every
    output row. r34 only depends on 32 rows of x, so the HOST computes it
    exactly and ships it as a per-core [128, 8] bias table. Heads 3/4 then
    vanish from the device program entirely.
  - matmul1 (v = x @ W_in.T): fp8e4 DoubleRow with a dual-plane
    error-compensated split: x = xh + xl, 64*W = Wh + Wl (same scale for all
    planes), accumulate xh@Wh + xh@Wl + xl@Wh in one PSUM group; the dropped
    xl@Wl term and plane-residuals are ~0.2% — bf16-level accuracy at half
    the PE cost. The 1/64 descale rides the PSUM->SBUF copy for free.
  - attention: att^T_h = v_h.T @ A_h^T as banded bf16 matmuls: stationary =
    v 128x128 slices, moving = analytic A^T window blocks (host-precomputed
    exact softmax weights, truncated at |key-center| <= 4). All interior tiles
    share ONE shift-invariant [128, 136] window table; boundary tiles get
    exact renormalized tables. Windows split at PSUM-bank (512 col)
    boundaries and accumulate via per-element has_written bits. Attention
    column-groups are emitted inside the phase-1 tile loop as soon as their
    v tiles exist, so their PSUM->SBUF copies hide under phase-1 matmuls.
  - matmul2 computes out^T (feature-major): stationary = W_out^T slices,
    moving = att^T q-chunks, accumulated over the 6 banded heads — also in
    dual-plane fp8 DoubleRow (att planes are split on-device: hi = 8*att via
    one copy, lo = (8*att - hi) via one scalar_tensor_tensor). The r34 bias
    and the 1/(8*64) descale ride the PSUM->SBUF copy. Output leaves the
    device as bf16 out^T; the host transposes and upcasts.
  - PE p-state: dummy matmuls on a zeroed scratch tile run during the initial
    DMA fill so the clock ramp completes before real work arrives.
"""

import os
import sys
import numpy as np

for _p in ("/opt/trn_rl_repo",):
    if _p not in sys.path:
        sys.path.insert(0, _p)

import concourse.bass as bass
import concourse.bacc as bacc
import concourse.mybir as mybir
from concourse import tile
from concourse import bass2jax as _b2j

# ---------------- problem constants (hardcoded per contract) ----------------
B = 8
L = 2048
E = 1024
H = 8
P = 128
SIGMA = 1.0
DISP = 1
NT = L // 128           # 16 seq tiles
R = 4                   # Gaussian taps kept each side of the center
WIN = 128 + 2 * R       # 136: per-tile attention query window
DT = mybir.dt.float32
BF = mybir.dt.bfloat16
F8 = mybir.dt.float8e4

GEMM1_FP8 = os.environ.get("K_GEMM1_FP8", "1") == "1"
GEMM2_FP8 = os.environ.get("K_GEMM2_FP8", "1") == "1"
W_SCALE = 64.0          # fp8 plane scale for W_in / W_out
ATT_SCALE = 8.0         # fp8 plane scale for att^T

BANDED_HEADS = [0, 1, 2, 5, 6, 7]   # center,left,right,center,left,right
NBH = len(BANDED_HEADS)
BI_TYPE = [0, 1, 2, 0, 1, 2]        # 0=center,1=left,2=right
TYPE_DISP = [0, -DISP, +DISP]
NQ4 = L // 512                      # 4 attention PSUM column groups
# emit attention (q4 group, head pair) at the end of phase-1 tile iteration i
ATT_EMIT_AT = {
    5: (0, 0), 6: (0, 1), 7: (0, 2),
    9: (1, 0), 10: (1, 1), 11: (1, 2),
    12: (2, 0), 13: (2, 1), 14: (2, 2),
    15: (3, 0),
}


def _g(x):
    return np.exp(-(np.asarray(x, dtype=np.float64) ** 2) / (2.0 * SIGMA**2))


def _attn_tables():
    """[128, 7*136] float: interior | first(t=0..2) | last(t=0..2) A^T blocks.

    interior[k, c] = g(k + R - c)/Zinf  (shift-invariant, shared by all head
    types and tiles 1..14; the head displacement only moves the window).
    first/last blocks are exact full-softmax values at the sequence edges.
    """
    tab = np.zeros((128, 7 * WIN), dtype=np.float64)
    zinf = _g(np.arange(-64, 65)).sum()
    k = np.arange(128)
    c = np.arange(WIN)
    delta = k[:, None] + R - c[None, :]
    tab[:, 0:WIN] = np.where(np.abs(delta) <= R, _g(delta) / zinf, 0.0)

    j = np.arange(L, dtype=np.float64)
    for t in range(3):
        d = TYPE_DISP[t]
        # first block: tile 0, queries q in [0, 132 - d)
        w = 132 - d
        q = np.arange(w, dtype=np.float64)
        logits = _g(j[None, :] - (q[:, None] + d))          # [w, L]
        A = logits / logits.sum(axis=1, keepdims=True)
        tab[:, (1 + t) * WIN : (1 + t) * WIN + w] = A[:, 0:128].T
        # last block: tile 15, queries q in [1916 - d, 2048)
        w = 132 + d
        q = np.arange(L - w, L, dtype=np.float64)
        logits = _g(j[None, :] - (q[:, None] + d))
        A = logits / logits.sum(axis=1, keepdims=True)
        tab[:, (4 + t) * WIN : (4 + t) * WIN + w] = A[:, L - 128 : L].T
    return tab


def _attn_pieces():
    """pieces[t][q4] = ordered [(tile i, col within 512-psum, width, table col)]."""
    pieces = [[[] for _ in range(NQ4)] for _ in range(3)]
    for t in range(3):
        d = TYPE_DISP[t]
        for i in range(NT):
            if i == 0:
                w0, w, base = 0, 132 - d, (1 + t) * WIN
            elif i == NT - 1:
                w0, w, base = 128 * i - R - d, 132 + d, (4 + t) * WIN
            else:
                w0, w, base = 128 * i - R - d, WIN, 0
            p0 = w0
            while p0 < w0 + w:
                q4 = p0 // 512
                pend = min(w0 + w, (q4 + 1) * 512)
                pieces[t][q4].append((i, p0 - 512 * q4, pend - p0, base + p0 - w0))
                p0 = pend
    return pieces


ATT_PIECES = _attn_pieces()

# GEMM1 column chunks: banded heads only (0-2 -> [0,384), 5-7 -> [640,1024))
G1_CHUNKS = ((0, 384), (640, 384))


def _build_program(phases=3):
    nc = bacc.Bacc("TRN2", target_bir_lowering=False, debug=False, num_devices=B)

    # w_in ships only the 6 banded heads' 6144 output columns, m-half-major:
    # [mh][kc/kt][(i)][384] so each m-half is one contiguous early DMA.
    if GEMM1_FP8:
        xt8 = nc.dram_tensor("xt8", [L, 2 * E], F8, kind="ExternalInput")
        w_in_hi = nc.dram_tensor("w_in_hi", [128, 6144], F8, kind="ExternalInput")
        w_in_lo = nc.dram_tensor("w_in_lo", [128, 6144], F8, kind="ExternalInput")
    else:
        xt = nc.dram_tensor("xt", [L, E], BF, kind="ExternalInput")
        w_in = nc.dram_tensor("w_in", [128, 6144], BF, kind="ExternalInput")
    if GEMM2_FP8:
        w_out_hi = nc.dram_tensor("w_out_hi", [128, NBH * E], F8, kind="ExternalInput")
        w_out_lo = nc.dram_tensor("w_out_lo", [128, NBH * E], F8, kind="ExternalInput")
    else:
        w_out = nc.dram_tensor("w_out", [128, 8 * E], BF, kind="ExternalInput")
    a_tab = nc.dram_tensor("a_tab", [128, 7 * WIN], BF, kind="ExternalInput")
    r34t = nc.dram_tensor("r34t", [128, 8], DT, kind="ExternalInput")
    outT = nc.dram_tensor("outT", [E, L], BF, kind="ExternalOutput")

    with tile.TileContext(nc) as tc:
        with (
            tc.tile_pool(name="const", bufs=1) as cpool,
            tc.tile_pool(name="vbuf", bufs=1) as vpool,
            tc.tile_pool(name="outp", bufs=4) as outpool,
            tc.tile_pool(name="ps8", bufs=8, space="PSUM") as ps8,
        ):
            if GEMM2_FP8:
                wo_hi_sb = cpool.tile([128, NBH * E], F8, tag="wo_hi_sb")
                wo_lo_sb = cpool.tile([128, NBH * E], F8, tag="wo_lo_sb")
                att_hi = cpool.tile([128, 2 * NBH * E], F8, tag="att_hi")
                att_lo = cpool.tile([128, 2 * NBH * E], F8, tag="att_lo")
            else:
                w_out_sb = cpool.tile([128, 8 * E], BF, tag="w_out_sb")
                att_sb = cpool.tile([128, 2 * NBH * E], BF, tag="att_sb")
            a_sb = cpool.tile([128, 7 * WIN], BF, tag="a_sb")
            r34_sb = cpool.tile([128, 8], DT, tag="r34_sb")
            scratch = cpool.tile([128, 512], BF, tag="scratch")
            v_sb = vpool.tile([128, NT * E], BF, tag="v_sb")

            # ---- PE warmup: ramp the p-state during the DMA fill ----
            nc.vector.memset(scratch[:, 0:128], 0.0)
            nc.vector.memset(scratch[:, 128:512], 0.0)
            pw = ps8.tile([128, 512], DT, tag="ps")
            for _ in range(3):
                nc.tensor.matmul(
                    pw[:, 0:128], scratch[:, 0:128], scratch[:, 0:128],
                    start=True, stop=True,
                )
            for _ in range(8):
                nc.tensor.matmul(
                    pw[:], scratch[:, 0:128], scratch[:, 0:512],
                    start=True, stop=True,
                )

            # attention helpers -------------------------------------------
            copy_rr = [0]

            def emit_att_group(q4, pair=None):
                heads = list(enumerate(BANDED_HEADS))
                if pair is not None:
                    heads = heads[2 * pair : 2 * pair + 2]
                for bi, h in heads:
                    ms = ATT_PIECES[BI_TYPE[bi]][q4]
                    patt_t = ps8.tile([128, 512], DT, tag="ps")
                    patt = patt_t[:]
                    for n_, (i, col, wd, tcol) in enumerate(ms):
                        nc.tensor.matmul(
                            patt[:, col : col + wd],
                            v_sb[:, i * E + h * 128 : i * E + (h + 1) * 128],
                            a_sb[:, tcol : tcol + wd],
                            start=(n_ == 0),
                            stop=(n_ == len(ms) - 1),
                        )
                    s, qq = q4 // 2, q4 % 2
                    if GEMM2_FP8:
                        c, j = bi // 2, bi % 2
                        base = ((s * 3 + c) * 2 + j) * E + qq * 512
                        hi_dst = att_hi[:, base : base + 512]
                        lo_dst = att_lo[:, base : base + 512]
                        nc.scalar.activation(
                            hi_dst, patt,
                            mybir.ActivationFunctionType.Copy,
                            scale=ATT_SCALE,
                        )
                        nc.vector.scalar_tensor_tensor(
                            lo_dst, patt, ATT_SCALE, hi_dst,
                            mybir.AluOpType.mult, mybir.AluOpType.subtract,
                        )
                    else:
                        dst = att_sb[:, (s * NBH + bi) * E + qq * 512 :][:, :512]
                        if copy_rr[0] % 2 == 0:
                            nc.scalar.copy(dst, patt)
                        else:
                            nc.vector.tensor_copy(dst, patt)
                    copy_rr[0] += 1

            # ---- phase 1: v = x @ W_in.T (banded-head columns only) ----
            with (
                tc.tile_pool(name="w_in_p", bufs=1) as wpool,
                tc.tile_pool(name="xt_p", bufs=7) as xtpool,
            ):
                if GEMM1_FP8:
                    w_hi_sb = wpool.tile([128, 6144], F8, tag="w_hi_sb")
                    w_lo_sb = wpool.tile([128, 6144], F8, tag="w_lo_sb")

                    def load_xt(i):
                        t8 = xtpool.tile([128, 2 * E], F8, tag="xt")
                        nc.sync.dma_start(t8[:], xt8[i * 128 : (i + 1) * 128, :])
                        return t8[:, 0:E], t8[:, E : 2 * E]

                    def load_w(mh):
                        nc.sync.dma_start(
                            w_hi_sb[:, mh * 3072 : (mh + 1) * 3072],
                            w_in_hi[:, mh * 3072 : (mh + 1) * 3072],
                        )
                        nc.sync.dma_start(
                            w_lo_sb[:, mh * 3072 : (mh + 1) * 3072],
                            w_in_lo[:, mh * 3072 : (mh + 1) * 3072],
                        )
                else:
                    w_in_sb = wpool.tile([128, 6144], BF, tag="w_in_sb")

                    def load_xt(i):
                        t_ = xtpool.tile([128, E], BF, tag="xt")
                        nc.sync.dma_start(t_[:], xt[i * 128 : (i + 1) * 128, :])
                        return t_

                    def load_w(mh):
                        nc.sync.dma_start(
                            w_in_sb[:, mh * 3072 : (mh + 1) * 3072],
                            w_in[:, mh * 3072 : (mh + 1) * 3072],
                        )

                # stream in consumption order; xt0 first (w's sem lands last)
                xts = [load_xt(0)]
                load_w(0)
                xts.append(load_xt(1))
                load_w(1)
                xts.append(load_xt(2))
                xts.append(load_xt(3))

                pvs = {}
                vsc = (1.0 / W_SCALE) if GEMM1_FP8 else 1.0

                def g1_chunk(i, mh):
                    m0, n = G1_CHUNKS[mh]
                    pvc = ps8.tile([128, 512], DT, tag="ps")
                    pvs[(i, mh)] = pvc
                    pv = pvc
                    if GEMM1_FP8:
                        th, tl = xts[i]
                        for kc in range(4):
                            sh = th[:, kc * 256 : (kc + 1) * 256].rearrange(
                                "p (i l) -> p i l", i=2
                            )
                            sl = tl[:, kc * 256 : (kc + 1) * 256].rearrange(
                                "p (i l) -> p i l", i=2
                            )
                            wh = w_hi_sb[
                                :, (mh * 4 + kc) * 768 : (mh * 4 + kc + 1) * 768
                            ].rearrange("p (i m) -> p i m", i=2)
                            wl = w_lo_sb[
                                :, (mh * 4 + kc) * 768 : (mh * 4 + kc + 1) * 768
                            ].rearrange("p (i m) -> p i m", i=2)
                            for term, (s_, m_) in enumerate(
                                ((sh, wh), (sh, wl), (sl, wh))
                            ):
                                nc.tensor.matmul(
                                    pv[:, 0:n],
                                    s_,
                                    m_,
                                    start=(kc == 0 and term == 0),
                                    stop=(kc == 3 and term == 2),
                                    perf_mode=mybir.MatmulPerfMode.DoubleRow,
                                )
                    else:
                        xt_t = xts[i]
                        for kt in range(8):
                            nc.tensor.matmul(
                                pv[:, 0:n],
                                xt_t[:, kt * 128 : (kt + 1) * 128],
                                w_in_sb[
                                    :, (mh * 8 + kt) * 384 : (mh * 8 + kt + 1) * 384
                                ],
                                start=(kt == 0),
                                stop=(kt == 7),
                            )

                def g1_copy(i, mh):
                    m0, n = G1_CHUNKS[mh]
                    pv = pvs.pop((i, mh))
                    dst = v_sb[:, i * E + m0 : i * E + m0 + n]
                    if (i + mh) % 2 == 0:
                        nc.scalar.activation(
                            dst, pv[:, 0:n],
                            mybir.ActivationFunctionType.Copy, scale=vsc,
                        )
                    else:
                        if GEMM1_FP8:
                            nc.vector.tensor_scalar_mul(dst, pv[:, 0:n], vsc)
                        else:
                            nc.vector.tensor_copy(dst, pv[:, 0:n])

                # tiles 0/1 interleave m-halves so PE work tracks DMA arrival
                for i_, mh_ in ((0, 0), (1, 0), (0, 1), (1, 1)):
                    g1_chunk(i_, mh_)
                    g1_copy(i_, mh_)

                xts.append(load_xt(4))
                xts.append(load_xt(5))
                nc.sync.dma_start(a_sb[:], a_tab[:])
                nc.sync.dma_start(r34_sb[:], r34t[:])

                for i in range(2, NT):
                    if i + 4 < NT:
                        xts.append(load_xt(i + 4))
                    if GEMM2_FP8:
                        if 4 <= i < 7:
                            c = i - 4
                            nc.sync.dma_start(
                                wo_hi_sb[:, c * 2048 : (c + 1) * 2048],
                                w_out_hi[:, c * 2048 : (c + 1) * 2048],
                            )
                            nc.sync.dma_start(
                                wo_lo_sb[:, c * 2048 : (c + 1) * 2048],
                                w_out_lo[:, c * 2048 : (c + 1) * 2048],
                            )
                    else:
                        if 4 <= i < 12:
                            c = i - 4
                            nc.sync.dma_start(
                                w_out_sb[:, c * 1024 : (c + 1) * 1024],
                                w_out[:, c * 1024 : (c + 1) * 1024],
                            )
                    for mh_ in (0, 1):
                        g1_chunk(i, mh_)
                    for mh_ in (0, 1):
                        g1_copy(i, mh_)
                    if i in ATT_EMIT_AT:
                        q4_, pair_ = ATT_EMIT_AT[i]
                        emit_att_group(q4_, pair_)

            emit_att_group(3, 1)
            emit_att_group(3, 2)

            # ---- phase 3: out^T = W_out @ att^T + r34 bias ----
            osc = 1.0 / (ATT_SCALE * W_SCALE)
            nout = 0
            for qc in range(4):
                s, qq = qc // 2, qc % 2
                for et in range(8):
                    po_t = ps8.tile([128, 512], DT, tag="ps")
                    po = po_t[:]
                    # final chunk splits so the very last output DMA is tiny;
                    # sub-chunk 2 gets its own PSUM bank so its matmuls don't
                    # serialize behind sub-chunk 1's PSUM read (bank tracker)
                    last = qc == 3 and et == 7
                    subs = ((0, 384), (384, 128)) if last else ((0, 512),)
                    for si, (s0, sn) in enumerate(subs):
                        if si == 1:
                            po_t2 = ps8.tile([128, 512], DT, tag="ps")
                            po = po_t2[:]
                        p0 = 0 if si == 1 else s0
                        if GEMM2_FP8:
                            nmm = 0
                            for c in range(3):
                                rh = att_hi[
                                    :, (s * 3 + c) * 2048 :][:, :2048].rearrange(
                                    "p (j q) -> p j q", j=2
                                )[:, :, qq * 512 + s0 : qq * 512 + s0 + sn]
                                rl = att_lo[
                                    :, (s * 3 + c) * 2048 :][:, :2048].rearrange(
                                    "p (j q) -> p j q", j=2
                                )[:, :, qq * 512 + s0 : qq * 512 + s0 + sn]
                                lh = wo_hi_sb[
                                    :, c * 2048 : (c + 1) * 2048
                                ].rearrange("p (j e) -> p j e", j=2)[
                                    :, :, et * 128 : (et + 1) * 128
                                ]
                                ll = wo_lo_sb[
                                    :, c * 2048 : (c + 1) * 2048
                                ].rearrange("p (j e) -> p j e", j=2)[
                                    :, :, et * 128 : (et + 1) * 128
                                ]
                                for s_, m_ in ((lh, rh), (ll, rh), (lh, rl)):
                                    nc.tensor.matmul(
                                        po[:, p0 : p0 + sn], s_, m_,
                                        start=(nmm == 0),
                                        stop=(nmm == 8),
                                        perf_mode=mybir.MatmulPerfMode.DoubleRow,
                                    )
                                    nmm += 1
                        else:
                            for bi, h in enumerate(BANDED_HEADS):
                                nc.tensor.matmul(
                                    po[:, p0 : p0 + sn],
                                    w_out_sb[
                                        :, h * E + et * 128 : h * E + (et + 1) * 128
                                    ],
                                    att_sb[
                                        :, (s * NBH + bi) * E + qq * 512 + s0 :
                                    ][:, :sn],
                                    start=(bi == 0),
                                    stop=(bi == NBH - 1),
                                )
                        ot = outpool.tile([128, 512], BF, tag="ot")
                        bias = r34_sb[:, et : et + 1]
                        if GEMM2_FP8:
                            if nout % 2 == 0:
                                nc.scalar.activation(
                                    ot[:, 0:sn], po[:, p0 : p0 + sn],
                                    mybir.ActivationFunctionType.Identity,
                                    bias=bias, scale=osc,
                                )
                            else:
                                nc.vector.tensor_scalar(
                                    ot[:, 0:sn], po[:, p0 : p0 + sn], osc, bias,
                                    mybir.AluOpType.mult, mybir.AluOpType.add,
                                )
                        else:
                            if nout % 2 == 0:
                                nc.scalar.add(ot[:, 0:sn], po[:, p0 : p0 + sn], bias)
                            else:
                                nc.vector.tensor_scalar_add(
                                    ot[:, 0:sn], po[:, p0 : p0 + sn], bias
                                )
                        nout += 1
                        nc.sync.dma_start(
                            outT[
                                et * 128 : (et + 1) * 128,
                                qc * 512 + s0 : qc * 512 + s0 + sn,
                            ],
                            ot[:, 0:sn],
                        )

    nc.compile()
    return nc


# ------------------------- host-side preparation ---------------------------

_NPBF = mybir.dt.np(BF)
_NPF8 = mybir.dt.np(F8)
# banded-head output columns of W_in^T, m-half-major (0:384 then 640:1024)
_MCOLS = np.concatenate([np.arange(0, 384), np.arange(640, 1024)])


def _host_wf_wl():
    """Exact 'first'/'last' head weight vectors over their 16-key support."""
    j = np.arange(L, dtype=np.float64)
    zf = _g(j - 0.0).sum()
    zl = _g(j - (L - 1.0)).sum()
    wf = _g(np.arange(16)) / zf
    wl = _g(np.arange(L - 16, L) - (L - 1.0)) / zl
    return wf, wl


def _host_r34(x, W_in, W_out):
    """[B, 128, 8] fp32: per-core output bias rows from the 'first'/'last'
    heads, computed exactly on the host (r34t[p, et] = r34[et*128 + p])."""
    wf, wl = _host_wf_wl()
    x64 = x.astype(np.float64)
    s3 = np.einsum("k,bke->be", wf, x64[:, 0:16, :])        # [B, E]
    s4 = np.einsum("k,bke->be", wl, x64[:, L - 16 : L, :])
    W_in64 = W_in.astype(np.float64)
    W_out64 = W_out.astype(np.float64)
    u3 = s3 @ W_in64.T[:, 384:512]                          # [B, 128]
    u4 = s4 @ W_in64.T[:, 512:640]
    r34 = u3 @ W_out64.T[384:512, :] + u4 @ W_out64.T[512:640, :]  # [B, E]
    return np.ascontiguousarray(
        r34.reshape(B, 8, 128).transpose(0, 2, 1)
    ).astype(np.float32)


def _pack_xt_bf16(x):
    # xt[b, i*128 + p, kt*128 + l] = x[b, i*128 + l, kt*128 + p]
    t = x.reshape(B, NT, 128, 8, 128).transpose(0, 1, 4, 3, 2)
    return np.ascontiguousarray(t).reshape(B * L, E).astype(_NPBF)


def _pack_xt_fp8(xq):
    # xt[b, i*128 + p, kc*256 + ipl*128 + l] = xq[b, i*128 + l, kc*256 + ipl*128 + p]
    t = xq.reshape(B, NT, 128, 4, 2, 128).transpose(0, 1, 5, 3, 4, 2)
    return np.ascontiguousarray(t).reshape(B * L, E)


def _pack_w_bf16(Wt):
    # w[p, (mh*8 + kt)*384 + m] = W.T[kt*128 + p, mcol(mh, m)]
    t = Wt.reshape(8, 128, E)[:, :, _MCOLS]          # [kt, p, mh*384+m]
    t = t.reshape(8, 128, 2, 384).transpose(1, 2, 0, 3)
    return np.ascontiguousarray(t).reshape(128, 6144).astype(_NPBF)


def _pack_w_fp8(Wq):
    # w[p, ((mh*4 + kc)*2 + ipl)*384 + m] = Wq[kc*256 + ipl*128 + p, mcol(mh, m)]
    t = Wq.reshape(4, 2, 128, E)[:, :, :, _MCOLS]    # [kc, ipl, p, mh*384+m]
    t = t.reshape(4, 2, 128, 2, 384).transpose(2, 3, 0, 1, 4)
    return np.ascontiguousarray(t).reshape(128, 6144)


def _pack_wo_fp8(Wq):
    # Wq: [NBH*128, E] rows = banded-head-major features (bi, p).
    # wo[p, c*2048 + j*1024 + e] = Wq[(c*2 + j)*128 + p, e]
    t = Wq.reshape(3, 2, 128, E).transpose(2, 0, 1, 3)
    return np.ascontiguousarray(t).reshape(128, NBH * E)


def _split_f8(a):
    hi = a.astype(_NPF8)
    lo = (a - hi.astype(np.float32)).astype(_NPF8)
    return hi, lo


class _Runner:
    """Builds the Bass program once and caches a jitted shard_map executable
    (one batch element per NeuronCore)."""

    def __init__(self):
        import jax
        from jax.sharding import Mesh, PartitionSpec
        from jax.experimental.shard_map import shard_map

        self.jax = jax
        _b2j.install_neuronx_cc_hook()
        nc = _build_program()
        self.nc = nc
        self.a_tab_np = _attn_tables().astype(_NPBF)

        partition_name = (
            nc.partition_id_tensor.name if nc.partition_id_tensor else None
        )
        in_names = []
        out_names = []
        out_avals = []
        for alloc in nc.m.functions[0].allocations:
            if not isinstance(alloc, mybir.MemoryLocationSet):
                continue
            name = alloc.memorylocations[0].name
            if alloc.kind == "ExternalInput":
                if name != partition_name:
                    in_names.append(name)
            elif alloc.kind == "ExternalOutput":
                out_names.append(name)
                out_avals.append(
                    jax.core.ShapedArray(
                        tuple(alloc.tensor_shape), mybir.dt.np(alloc.dtype)
                    )
                )
        self.in_names = in_names
        self.out_names = out_names
        self.out_avals = out_avals
        n_params = len(in_names)
        n_outs = len(out_names)
        all_names = tuple(in_names) + tuple(out_names)
        if partition_name is not None:
            all_names = all_names + (partition_name,)

        def _body(*args):
            operands = list(args)
            if partition_name is not None:
                operands.append(_b2j.partition_id_tensor())
            outs = _b2j._bass_exec_p.bind(
                *operands,
                out_avals=tuple(out_avals),
                in_names=all_names,
                out_names=tuple(out_names),
                lowering_input_output_aliases=(),
                sim_require_finite=True,
                sim_require_nnan=True,
                nc=nc,
            )
            return tuple(outs)

        devices = jax.devices()[:B]
        assert len(devices) == B
        self.mesh = Mesh(np.asarray(devices), ("core",))
        in_specs = (PartitionSpec("core"),) * (n_params + n_outs)
        out_specs = (PartitionSpec("core"),) * n_outs
        self.sharded = jax.jit(
            shard_map(
                _body,
                mesh=self.mesh,
                in_specs=in_specs,
                out_specs=out_specs,
                check_rep=False,
            ),
            donate_argnums=tuple(range(n_params, n_params + n_outs)),
            keep_unused=True,
        )

    def run_device(self, dev_args):
        jnp = self.jax.numpy
        zeros = [
            jnp.zeros((B * av.shape[0], *av.shape[1:]), av.dtype)
            for av in self.out_avals
        ]
        return self.sharded(*dev_args, *zeros)

    def prepare_inputs(self, x, W_in, W_out):
        jax = self.jax
        dev = {}
        if GEMM1_FP8:
            xh, xl = _split_f8(x)
            dev["xt8"] = np.concatenate(
                [_pack_xt_fp8(xh), _pack_xt_fp8(xl)], axis=1
            )
            Wt = np.ascontiguousarray(W_in.T) * np.float32(W_SCALE)
            Wh, Wl = _split_f8(Wt)
            dev["w_in_hi"] = np.concatenate([_pack_w_fp8(Wh)] * B, axis=0)
            dev["w_in_lo"] = np.concatenate([_pack_w_fp8(Wl)] * B, axis=0)
        else:
            dev["xt"] = _pack_xt_bf16(x)
            w_in_b = _pack_w_bf16(np.ascontiguousarray(W_in.T))
            dev["w_in"] = np.concatenate([w_in_b] * B, axis=0)
        if GEMM2_FP8:
            rows = np.concatenate(
                [np.arange(h * 128, (h + 1) * 128) for h in BANDED_HEADS]
            )
            Wq = np.ascontiguousarray(W_out.T[rows, :]) * np.float32(W_SCALE)
            Wh, Wl = _split_f8(Wq)
            dev["w_out_hi"] = np.concatenate([_pack_wo_fp8(Wh)] * B, axis=0)
            dev["w_out_lo"] = np.concatenate([_pack_wo_fp8(Wl)] * B, axis=0)
        else:
            w_out_b = _pack_w_bf16(np.ascontiguousarray(W_out.T))
            dev["w_out"] = np.concatenate([w_out_b] * B, axis=0)
        dev["a_tab"] = np.concatenate([self.a_tab_np] * B, axis=0)
        dev["r34t"] = _host_r34(x, W_in, W_out).reshape(B * 128, 8)
        return [jax.device_put(dev[name]) for name in self.in_names]

    def __call__(self, x, W_in, W_out):
        args = self.prepare_inputs(x, W_in, W_out)
        outs = self.run_device(args)
        outT = np.asarray(outs[self.out_names.index("outT")])
        # outT: [B*E, L] bf16 -> [B, L, E] fp32
        return np.ascontiguousarray(
            outT.reshape(B, E, L).transpose(0, 2, 1)
        ).astype(np.float32)


_CACHE = {}


def _get_runner() -> _Runner:
    if "runner" not in _CACHE:
        _CACHE["runner"] = _Runner()
    return _CACHE["runner"]


def kernel(x, W_in, W_out):
    x = np.ascontiguousarray(np.asarray(x, dtype=np.float32))
    W_in = np.ascontiguousarray(np.asarray(W_in, dtype=np.float32))
    W_out = np.ascontiguousarray(np.asarray(W_out, dtype=np.float32))
    assert x.shape == (B, L, E)
    return _get_runner()(x, W_in, W_out)


if __name__ == "__main__":
    rng = np.random.default_rng(0)
    x = rng.standard_normal((B, L, E), dtype=np.float32)
    W_in = rng.standard_normal((E, E), dtype=np.float32) * 0.05
    W_out = rng.standard_normal((E, E), dtype=np.float32) * 0.05
    y = kernel(x, W_in, W_out)
    print("out", y.shape, y.dtype, np.abs(y).mean())


# revision 39
# speedup vs baseline: 1.5331x; 1.0096x over previous
"""Trainium2 Bass kernel for nn_NewAttention (analytic Gaussian sparse attention).

Math (per batch element b):
    v        = x[b] @ W_in.T                      # [L, E]
    per head h (P=128 cols of v):
        A_h  = softmax(-(j - c_h(i))^2 / 2)       # [L, L], analytic, banded
        att_h = A_h @ v_h                         # [L, P]
    out[b]   = concat_h(att_h) @ W_out.T          # [L, E]

Sharding: data-parallel over batch, one batch element per NeuronCore (8 cores).

Device strategy (per core):
  - 'first'/'last' heads (3/4) attend to a fixed key location for every query,
    so their output contribution is a single row vector r34[e] added to every
    output row. r34 only depends on 32 rows of x, so the HOST computes it
    exactly and ships it as a per-core [128, 8] bias table. Heads 3/4 then
    vanish from the device program entirely.
  - matmul1 (v = x @ W_in.T): fp8e4 DoubleRow with a dual-plane
    error-compensated split: x = xh + xl, 64*W = Wh + Wl (same scale for all
    planes), accumulate xh@Wh + xh@Wl + xl@Wh in one PSUM group; the dropped
    xl@Wl term and plane-residuals are ~0.2% — bf16-level accuracy at half
    the PE cost. The 1/64 descale rides the PSUM->SBUF copy for free.
  - attention: att^T_h = v_h.T @ A_h^T as banded bf16 matmuls: stationary =
    v 128x128 slices, moving = analytic A^T window blocks (host-precomputed
    exact softmax weights, truncated at |key-center| <= 4). All interior tiles
    share ONE shift-invariant [128, 136] window table; boundary tiles get
    exact renormalized tables. Windows split at PSUM-bank (512 col)
    boundaries and accumulate via per-element has_written bits. Attention
    column-groups are emitted inside the phase-1 tile loop as soon as their
    v tiles exist, so their PSUM->SBUF copies hide under phase-1 matmuls.
  - matmul2 computes out^T (feature-major): stationary = W_out^T slices,
    moving = att^T q-chunks, accumulated over the 6 banded heads — also in
    dual-plane fp8 DoubleRow (att planes are split on-device: hi = 8*att via
    one copy, lo = (8*att - hi) via one scalar_tensor_tensor). The r34 bias
    and the 1/(8*64) descale ride the PSUM->SBUF copy. Output leaves the
    device as bf16 out^T; the host transposes and upcasts.
  - PE p-state: dummy matmuls on a zeroed scratch tile run during the initial
    DMA fill so the clock ramp completes before real work arrives.
"""

import os
import sys
import numpy as np

for _p in ("/opt/trn_rl_repo",):
    if _p not in sys.path:
        sys.path.insert(0, _p)

import concourse.bass as bass
import concourse.bacc as bacc
import concourse.mybir as mybir
from concourse import tile
from concourse import bass2jax as _b2j

# ---------------- problem constants (hardcoded per contract) ----------------
B = 8
L = 2048
E = 1024
H = 8
P = 128
SIGMA = 1.0
DISP = 1
NT = L // 128           # 16 seq tiles
R = 4                   # Gaussian taps kept each side of the center
WIN = 128 + 2 * R       # 136: per-tile attention query window
DT = mybir.dt.float32
BF = mybir.dt.bfloat16
F8 = mybir.dt.float8e4

GEMM1_FP8 = os.environ.get("K_GEMM1_FP8", "1") == "1"
GEMM2_FP8 = os.environ.get("K_GEMM2_FP8", "1") == "1"
W_SCALE = 64.0          # fp8 plane scale for W_in / W_out
ATT_SCALE = 8.0         # fp8 plane scale for att^T

BANDED_HEADS = [0, 1, 2, 5, 6, 7]   # center,left,right,center,left,right
NBH = len(BANDED_HEADS)
BI_TYPE = [0, 1, 2, 0, 1, 2]        # 0=center,1=left,2=right
TYPE_DISP = [0, -DISP, +DISP]
NQ4 = L // 512                      # 4 attention PSUM column groups
# emit attention (q4 group, head pair) at the end of phase-1 tile iteration i
ATT_EMIT_AT = {
    5: (0, 0), 6: (0, 1), 7: (0, 2),
    9: (1, 0), 10: (1, 1), 11: (1, 2),
    12: (2, 0), 13: (2, 1), 14: (2, 2),
    15: (3, 0),
}


def _g(x):
    return np.exp(-(np.asarray(x, dtype=np.float64) ** 2) / (2.0 * SIGMA**2))


def _attn_tables():
    """[128, 7*136] float: interior | first(t=0..2) | last(t=0..2) A^T blocks.

    interior[k, c] = g(k + R - c)/Zinf  (shift-invariant, shared by all head
    types and tiles 1..14; the head displacement only moves the window).
    first/last blocks are exact full-softmax values at the sequence edges.
    """
    tab = np.zeros((128, 7 * WIN), dtype=np.float64)
    zinf = _g(np.arange(-64, 65)).sum()
    k = np.arange(128)
    c = np.arange(WIN)
    delta = k[:, None] + R - c[None, :]
    tab[:, 0:WIN] = np.where(np.abs(delta) <= R, _g(delta) / zinf, 0.0)

    j = np.arange(L, dtype=np.float64)
    for t in range(3):
        d = TYPE_DISP[t]
        # first block: tile 0, queries q in [0, 132 - d)
        w = 132 - d
        q = np.arange(w, dtype=np.float64)
        logits = _g(j[None, :] - (q[:, None] + d))          # [w, L]
        A = logits / logits.sum(axis=1, keepdims=True)
        tab[:, (1 + t) * WIN : (1 + t) * WIN + w] = A[:, 0:128].T
        # last block: tile 15, queries q in [1916 - d, 2048)
        w = 132 + d
        q = np.arange(L - w, L, dtype=np.float64)
        logits = _g(j[None, :] - (q[:, None] + d))
        A = logits / logits.sum(axis=1, keepdims=True)
        tab[:, (4 + t) * WIN : (4 + t) * WIN + w] = A[:, L - 128 : L].T
    return tab


def _attn_pieces():
    """pieces[t][q4] = ordered [(tile i, col within 512-psum, width, table col)]."""
    pieces = [[[] for _ in range(NQ4)] for _ in range(3)]
    for t in range(3):
        d = TYPE_DISP[t]
        for i in range(NT):
            if i == 0:
                w0, w, base = 0, 132 - d, (1 + t) * WIN
            elif i == NT - 1:
                w0, w, base = 128 * i - R - d, 132 + d, (4 + t) * WIN
            else:
                w0, w, base = 128 * i - R - d, WIN, 0
            p0 = w0
            while p0 < w0 + w:
                q4 = p0 // 512
                pend = min(w0 + w, (q4 + 1) * 512)
                pieces[t][q4].append((i, p0 - 512 * q4, pend - p0, base + p0 - w0))
                p0 = pend
    return pieces


ATT_PIECES = _attn_pieces()

# GEMM1 column chunks: banded heads only (0-2 -> [0,384), 5-7 -> [640,1024))
G1_CHUNKS = ((0, 384), (640, 384))


def _build_program(phases=3):
    nc = bacc.Bacc("TRN2", target_bir_lowering=False, debug=False, num_devices=B)

    # w_in ships only the 6 banded heads' 6144 output columns, m-half-major:
    # [mh][kc/kt][(i)][384] so each m-half is one contiguous early DMA.
    if GEMM1_FP8:
        xt8 = nc.dram_tensor("xt8", [L, 2 * E], F8, kind="ExternalInput")
        w_in_hi = nc.dram_tensor("w_in_hi", [128, 6144], F8, kind="ExternalInput")
        w_in_lo = nc.dram_tensor("w_in_lo", [128, 6144], F8, kind="ExternalInput")
    else:
        xt = nc.dram_tensor("xt", [L, E], BF, kind="ExternalInput")
        w_in = nc.dram_tensor("w_in", [128, 6144], BF, kind="ExternalInput")
    if GEMM2_FP8:
        w_out_hi = nc.dram_tensor("w_out_hi", [128, NBH * E], F8, kind="ExternalInput")
        w_out_lo = nc.dram_tensor("w_out_lo", [128, NBH * E], F8, kind="ExternalInput")
    else:
        w_out = nc.dram_tensor("w_out", [128, 8 * E], BF, kind="ExternalInput")
    a_tab = nc.dram_tensor("a_tab", [128, 7 * WIN], BF, kind="ExternalInput")
    r34t = nc.dram_tensor("r34t", [128, 8], DT, kind="ExternalInput")
    outT = nc.dram_tensor("outT", [E, L], BF, kind="ExternalOutput")

    with tile.TileContext(nc) as tc:
        with (
            tc.tile_pool(name="const", bufs=1) as cpool,
            tc.tile_pool(name="vbuf", bufs=1) as vpool,
            tc.tile_pool(name="outp", bufs=4) as outpool,
            tc.tile_pool(name="ps8", bufs=8, space="PSUM") as ps8,
        ):
            if GEMM2_FP8:
                wo_hi_sb = cpool.tile([128, NBH * E], F8, tag="wo_hi_sb")
                wo_lo_sb = cpool.tile([128, NBH * E], F8, tag="wo_lo_sb")
                att_hi = cpool.tile([128, 2 * NBH * E], F8, tag="att_hi")
                att_lo = cpool.tile([128, 2 * NBH * E], F8, tag="att_lo")
            else:
                w_out_sb = cpool.tile([128, 8 * E], BF, tag="w_out_sb")
                att_sb = cpool.tile([128, 2 * NBH * E], BF, tag="att_sb")
            a_sb = cpool.tile([128, 7 * WIN], BF, tag="a_sb")
            r34_sb = cpool.tile([128, 8], DT, tag="r34_sb")
            scratch = cpool.tile([128, 512], BF, tag="scratch")
            v_sb = vpool.tile([128, NT * E], BF, tag="v_sb")

            # ---- PE warmup: ramp the p-state during the DMA fill ----
            nc.vector.memset(scratch[:, 0:128], 0.0)
            nc.vector.memset(scratch[:, 128:512], 0.0)
            pw = ps8.tile([128, 512], DT, tag="ps")
            for _ in range(3):
                nc.tensor.matmul(
                    pw[:, 0:128], scratch[:, 0:128], scratch[:, 0:128],
                    start=True, stop=True,
                )
            for _ in range(8):
                nc.tensor.matmul(
                    pw[:], scratch[:, 0:128], scratch[:, 0:512],
                    start=True, stop=True,
                )

            # attention helpers -------------------------------------------
            copy_rr = [0]

            def emit_att_group(q4, pair=None):
                heads = list(enumerate(BANDED_HEADS))
                if pair is not None:
                    heads = heads[2 * pair : 2 * pair + 2]
                for bi, h in heads:
                    ms = ATT_PIECES[BI_TYPE[bi]][q4]
                    patt_t = ps8.tile([128, 512], DT, tag="ps")
                    patt = patt_t[:]
                    for n_, (i, col, wd, tcol) in enumerate(ms):
                        nc.tensor.matmul(
                            patt[:, col : col + wd],
                            v_sb[:, i * E + h * 128 : i * E + (h + 1) * 128],
                            a_sb[:, tcol : tcol + wd],
                            start=(n_ == 0),
                            stop=(n_ == len(ms) - 1),
                        )
                    s, qq = q4 // 2, q4 % 2
                    if GEMM2_FP8:
                        c, j = bi // 2, bi % 2
                        base = ((s * 3 + c) * 2 + j) * E + qq * 512
                        hi_dst = att_hi[:, base : base + 512]
                        lo_dst = att_lo[:, base : base + 512]
                        nc.scalar.activation(
                            hi_dst, patt,
                            mybir.ActivationFunctionType.Copy,
                            scale=ATT_SCALE,
                        )
                        nc.vector.scalar_tensor_tensor(
                            lo_dst, patt, ATT_SCALE, hi_dst,
                            mybir.AluOpType.mult, mybir.AluOpType.subtract,
                        )
                    else:
                        dst = att_sb[:, (s * NBH + bi) * E + qq * 512 :][:, :512]
                        if copy_rr[0] % 2 == 0:
                            nc.scalar.copy(dst, patt)
                        else:
                            nc.vector.tensor_copy(dst, patt)
                    copy_rr[0] += 1

            # ---- phase 1: v = x @ W_in.T (banded-head columns only) ----
            with (
                tc.tile_pool(name="w_in_p", bufs=1) as wpool,
                tc.tile_pool(name="xt_p", bufs=7) as xtpool,
            ):
                if GEMM1_FP8:
                    w_hi_sb = wpool.tile([128, 6144], F8, tag="w_hi_sb")
                    w_lo_sb = wpool.tile([128, 6144], F8, tag="w_lo_sb")

                    def load_xt(i):
                        t8 = xtpool.tile([128, 2 * E], F8, tag="xt")
                        nc.sync.dma_start(t8[:], xt8[i * 128 : (i + 1) * 128, :])
                        return t8[:, 0:E], t8[:, E : 2 * E]

                    def load_w(mh):
                        for hf in range(2):
                            c0 = mh * 3072 + hf * 1536
                            nc.sync.dma_start(
                                w_hi_sb[:, c0 : c0 + 1536],
                                w_in_hi[:, c0 : c0 + 1536],
                            )
                            nc.sync.dma_start(
                                w_lo_sb[:, c0 : c0 + 1536],
                                w_in_lo[:, c0 : c0 + 1536],
                            )
                else:
                    w_in_sb = wpool.tile([128, 6144], BF, tag="w_in_sb")

                    def load_xt(i):
                        t_ = xtpool.tile([128, E], BF, tag="xt")
                        nc.sync.dma_start(t_[:], xt[i * 128 : (i + 1) * 128, :])
                        return t_

                    def load_w(mh):
                        nc.sync.dma_start(
                            w_in_sb[:, mh * 3072 : (mh + 1) * 3072],
                            w_in[:, mh * 3072 : (mh + 1) * 3072],
                        )

                # stream in consumption order; xt0 first (w's sem lands last)
                xts = [load_xt(0)]
                load_w(0)
                xts.append(load_xt(1))
                load_w(1)
                xts.append(load_xt(2))
                xts.append(load_xt(3))

                pvs = {}
                vsc = (1.0 / W_SCALE) if GEMM1_FP8 else 1.0

                def g1_chunk(i, mh):
                    m0, n = G1_CHUNKS[mh]
                    pvc = ps8.tile([128, 512], DT, tag="ps")
                    pvs[(i, mh)] = pvc
                    pv = pvc
                    if GEMM1_FP8:
                        th, tl = xts[i]
                        for kc in range(4):
                            sh = th[:, kc * 256 : (kc + 1) * 256].rearrange(
                                "p (i l) -> p i l", i=2
                            )
                            sl = tl[:, kc * 256 : (kc + 1) * 256].rearrange(
                                "p (i l) -> p i l", i=2
                            )
                            wh = w_hi_sb[
                                :, (mh * 4 + kc) * 768 : (mh * 4 + kc + 1) * 768
                            ].rearrange("p (i m) -> p i m", i=2)
                            wl = w_lo_sb[
                                :, (mh * 4 + kc) * 768 : (mh * 4 + kc + 1) * 768
                            ].rearrange("p (i m) -> p i m", i=2)
                            for term, (s_, m_) in enumerate(
                                ((sh, wh), (sh, wl), (sl, wh))
                            ):
                                nc.tensor.matmul(
                                    pv[:, 0:n],
                                    s_,
                                    m_,
                                    start=(kc == 0 and term == 0),
                                    stop=(kc == 3 and term == 2),
                                    perf_mode=mybir.MatmulPerfMode.DoubleRow,
                                )
                    else:
                        xt_t = xts[i]
                        for kt in range(8):
                            nc.tensor.matmul(
                                pv[:, 0:n],
                                xt_t[:, kt * 128 : (kt + 1) * 128],
                                w_in_sb[
                                    :, (mh * 8 + kt) * 384 : (mh * 8 + kt + 1) * 384
                                ],
                                start=(kt == 0),
                                stop=(kt == 7),
                            )

                def g1_copy(i, mh):
                    m0, n = G1_CHUNKS[mh]
                    pv = pvs.pop((i, mh))
                    dst = v_sb[:, i * E + m0 : i * E + m0 + n]
                    if (i + mh) % 2 == 0:
                        nc.scalar.activation(
                            dst, pv[:, 0:n],
                            mybir.ActivationFunctionType.Copy, scale=vsc,
                        )
                    else:
                        if GEMM1_FP8:
                            nc.vector.tensor_scalar_mul(dst, pv[:, 0:n], vsc)
                        else:
                            nc.vector.tensor_copy(dst, pv[:, 0:n])

                # tiles 0/1 interleave m-halves so PE work tracks DMA arrival
                for i_, mh_ in ((0, 0), (1, 0), (0, 1), (1, 1)):
                    g1_chunk(i_, mh_)
                    g1_copy(i_, mh_)

                xts.append(load_xt(4))
                xts.append(load_xt(5))
                nc.sync.dma_start(a_sb[:], a_tab[:])
                nc.sync.dma_start(r34_sb[:], r34t[:])

                for i in range(2, NT):
                    if i + 4 < NT:
                        xts.append(load_xt(i + 4))
                    if GEMM2_FP8:
                        if 4 <= i < 7:
                            c = i - 4
                            nc.sync.dma_start(
                                wo_hi_sb[:, c * 2048 : (c + 1) * 2048],
                                w_out_hi[:, c * 2048 : (c + 1) * 2048],
                            )
                            nc.sync.dma_start(
                                wo_lo_sb[:, c * 2048 : (c + 1) * 2048],
                                w_out_lo[:, c * 2048 : (c + 1) * 2048],
                            )
                    else:
                        if 4 <= i < 12:
                            c = i - 4
                            nc.sync.dma_start(
                                w_out_sb[:, c * 1024 : (c + 1) * 1024],
                                w_out[:, c * 1024 : (c + 1) * 1024],
                            )
                    for mh_ in (0, 1):
                        g1_chunk(i, mh_)
                    for mh_ in (0, 1):
                        g1_copy(i, mh_)
                    if i in ATT_EMIT_AT:
                        q4_, pair_ = ATT_EMIT_AT[i]
                        emit_att_group(q4_, pair_)

            emit_att_group(3, 1)
            emit_att_group(3, 2)

            # ---- phase 3: out^T = W_out @ att^T + r34 bias ----
            osc = 1.0 / (ATT_SCALE * W_SCALE)
            nout = 0
            for qc in range(4):
                s, qq = qc // 2, qc % 2
                for et in range(8):
                    po_t = ps8.tile([128, 512], DT, tag="ps")
                    po = po_t[:]
                    # final chunk splits so the very last output DMA is tiny;
                    # sub-chunk 2 gets its own PSUM bank so its matmuls don't
                    # serialize behind sub-chunk 1's PSUM read (bank tracker)
                    last = qc == 3 and et == 7
                    subs = ((0, 384), (384, 128)) if last else ((0, 512),)
                    for si, (s0, sn) in enumerate(subs):
                        if si == 1:
                            po_t2 = ps8.tile([128, 512], DT, tag="ps")
                            po = po_t2[:]
                        p0 = 0 if si == 1 else s0
                        if GEMM2_FP8:
                            nmm = 0
                            for c in range(3):
                                rh = att_hi[
                                    :, (s * 3 + c) * 2048 :][:, :2048].rearrange(
                                    "p (j q) -> p j q", j=2
                                )[:, :, qq * 512 + s0 : qq * 512 + s0 + sn]
                                rl = att_lo[
                                    :, (s * 3 + c) * 2048 :][:, :2048].rearrange(
                                    "p (j q) -> p j q", j=2
                                )[:, :, qq * 512 + s0 : qq * 512 + s0 + sn]
                                lh = wo_hi_sb[
                                    :, c * 2048 : (c + 1) * 2048
                                ].rearrange("p (j e) -> p j e", j=2)[
                                    :, :, et * 128 : (et + 1) * 128
                                ]
                                ll = wo_lo_sb[
                                    :, c * 2048 : (c + 1) * 2048
                                ].rearrange("p (j e) -> p j e", j=2)[
                                    :, :, et * 128 : (et + 1) * 128
                                ]
                                for s_, m_ in ((lh, rh), (ll, rh), (lh, rl)):
                                    nc.tensor.matmul(
                                        po[:, p0 : p0 + sn], s_, m_,
                                        start=(nmm == 0),
                                        stop=(nmm == 8),
                                        perf_mode=mybir.MatmulPerfMode.DoubleRow,
                                    )
                                    nmm += 1
                        else:
                            for bi, h in enumerate(BANDED_HEADS):
                                nc.tensor.matmul(
                                    po[:, p0 : p0 + sn],
                                    w_out_sb[
                                        :, h * E + et * 128 : h * E + (et + 1) * 128
                                    ],
                                    att_sb[
                                        :, (s * NBH + bi) * E + qq * 512 + s0 :
                                    ][:, :sn],
                                    start=(bi == 0),
                                    stop=(bi == NBH - 1),
                                )
                        ot = outpool.tile([128, 512], BF, tag="ot")
                        bias = r34_sb[:, et : et + 1]
                        use_act = (nout % 2 == 0) if not last else (si == 0)
                        if GEMM2_FP8:
                            if use_act:
                                nc.scalar.activation(
                                    ot[:, 0:sn], po[:, p0 : p0 + sn],
                                    mybir.ActivationFunctionType.Identity,
                                    bias=bias, scale=osc,
                                )
                            else:
                                nc.vector.tensor_scalar(
                                    ot[:, 0:sn], po[:, p0 : p0 + sn], osc, bias,
                                    mybir.AluOpType.mult, mybir.AluOpType.add,
                                )
                        else:
                            if use_act:
                                nc.scalar.add(ot[:, 0:sn], po[:, p0 : p0 + sn], bias)
                            else:
                                nc.vector.tensor_scalar_add(
                                    ot[:, 0:sn], po[:, p0 : p0 + sn], bias
                                )
                        nout += 1
                        dma_eng = nc.sync
                        if last and si == 0:
                            dma_eng = nc.scalar
                        dma_eng.dma_start(
                            outT[
                                et * 128 : (et + 1) * 128,
                                qc * 512 + s0 : qc * 512 + s0 + sn,
                            ],
                            ot[:, 0:sn],
                        )

    nc.compile()
    return nc


# ------------------------- host-side preparation ---------------------------

_NPBF = mybir.dt.np(BF)
_NPF8 = mybir.dt.np(F8)
# banded-head output columns of W_in^T, m-half-major (0:384 then 640:1024)
_MCOLS = np.concatenate([np.arange(0, 384), np.arange(640, 1024)])


def _host_wf_wl():
    """Exact 'first'/'last' head weight vectors over their 16-key support."""
    j = np.arange(L, dtype=np.float64)
    zf = _g(j - 0.0).sum()
    zl = _g(j - (L - 1.0)).sum()
    wf = _g(np.arange(16)) / zf
    wl = _g(np.arange(L - 16, L) - (L - 1.0)) / zl
    return wf, wl


def _host_r34(x, W_in, W_out):
    """[B, 128, 8] fp32: per-core output bias rows from the 'first'/'last'
    heads, computed exactly on the host (r34t[p, et] = r34[et*128 + p])."""
    wf, wl = _host_wf_wl()
    x64 = x.astype(np.float64)
    s3 = np.einsum("k,bke->be", wf, x64[:, 0:16, :])        # [B, E]
    s4 = np.einsum("k,bke->be", wl, x64[:, L - 16 : L, :])
    W_in64 = W_in.astype(np.float64)
    W_out64 = W_out.astype(np.float64)
    u3 = s3 @ W_in64.T[:, 384:512]                          # [B, 128]
    u4 = s4 @ W_in64.T[:, 512:640]
    r34 = u3 @ W_out64.T[384:512, :] + u4 @ W_out64.T[512:640, :]  # [B, E]
    return np.ascontiguousarray(
        r34.reshape(B, 8, 128).transpose(0, 2, 1)
    ).astype(np.float32)


def _pack_xt_bf16(x):
    # xt[b, i*128 + p, kt*128 + l] = x[b, i*128 + l, kt*128 + p]
    t = x.reshape(B, NT, 128, 8, 128).transpose(0, 1, 4, 3, 2)
    return np.ascontiguousarray(t).reshape(B * L, E).astype(_NPBF)


def _pack_xt_fp8(xq):
    # xt[b, i*128 + p, kc*256 + ipl*128 + l] = xq[b, i*128 + l, kc*256 + ipl*128 + p]
    t = xq.reshape(B, NT, 128, 4, 2, 128).transpose(0, 1, 5, 3, 4, 2)
    return np.ascontiguousarray(t).reshape(B * L, E)


def _pack_w_bf16(Wt):
    # w[p, (mh*8 + kt)*384 + m] = W.T[kt*128 + p, mcol(mh, m)]
    t = Wt.reshape(8, 128, E)[:, :, _MCOLS]          # [kt, p, mh*384+m]
    t = t.reshape(8, 128, 2, 384).transpose(1, 2, 0, 3)
    return np.ascontiguousarray(t).reshape(128, 6144).astype(_NPBF)


def _pack_w_fp8(Wq):
    # w[p, ((mh*4 + kc)*2 + ipl)*384 + m] = Wq[kc*256 + ipl*128 + p, mcol(mh, m)]
    t = Wq.reshape(4, 2, 128, E)[:, :, :, _MCOLS]    # [kc, ipl, p, mh*384+m]
    t = t.reshape(4, 2, 128, 2, 384).transpose(2, 3, 0, 1, 4)
    return np.ascontiguousarray(t).reshape(128, 6144)


def _pack_wo_fp8(Wq):
    # Wq: [NBH*128, E] rows = banded-head-major features (bi, p).
    # wo[p, c*2048 + j*1024 + e] = Wq[(c*2 + j)*128 + p, e]
    t = Wq.reshape(3, 2, 128, E).transpose(2, 0, 1, 3)
    return np.ascontiguousarray(t).reshape(128, NBH * E)


def _split_f8(a):
    hi = a.astype(_NPF8)
    lo = (a - hi.astype(np.float32)).astype(_NPF8)
    return hi, lo


class _Runner:
    """Builds the Bass program once and caches a jitted shard_map executable
    (one batch element per NeuronCore)."""

    def __init__(self):
        import jax
        from jax.sharding import Mesh, PartitionSpec
        from jax.experimental.shard_map import shard_map

        self.jax = jax
        _b2j.install_neuronx_cc_hook()
        nc = _build_program()
        self.nc = nc
        self.a_tab_np = _attn_tables().astype(_NPBF)

        partition_name = (
            nc.partition_id_tensor.name if nc.partition_id_tensor else None
        )
        in_names = []
        out_names = []
        out_avals = []
        for alloc in nc.m.functions[0].allocations:
            if not isinstance(alloc, mybir.MemoryLocationSet):
                continue
            name = alloc.memorylocations[0].name
            if alloc.kind == "ExternalInput":
                if name != partition_name:
                    in_names.append(name)
            elif alloc.kind == "ExternalOutput":
                out_names.append(name)
                out_avals.append(
                    jax.core.ShapedArray(
                        tuple(alloc.tensor_shape), mybir.dt.np(alloc.dtype)
                    )
                )
        self.in_names = in_names
        self.out_names = out_names
        self.out_avals = out_avals
        n_params = len(in_names)
        n_outs = len(out_names)
        all_names = tuple(in_names) + tuple(out_names)
        if partition_name is not None:
            all_names = all_names + (partition_name,)

        def _body(*args):
            operands = list(args)
            if partition_name is not None:
                operands.append(_b2j.partition_id_tensor())
            outs = _b2j._bass_exec_p.bind(
                *operands,
                out_avals=tuple(out_avals),
                in_names=all_names,
                out_names=tuple(out_names),
                lowering_input_output_aliases=(),
                sim_require_finite=True,
                sim_require_nnan=True,
                nc=nc,
            )
            return tuple(outs)

        devices = jax.devices()[:B]
        assert len(devices) == B
        self.mesh = Mesh(np.asarray(devices), ("core",))
        in_specs = (PartitionSpec("core"),) * (n_params + n_outs)
        out_specs = (PartitionSpec("core"),) * n_outs
        self.sharded = jax.jit(
            shard_map(
                _body,
                mesh=self.mesh,
                in_specs=in_specs,
                out_specs=out_specs,
                check_rep=False,
            ),
            donate_argnums=tuple(range(n_params, n_params + n_outs)),
            keep_unused=True,
        )

    def run_device(self, dev_args):
        jnp = self.jax.numpy
        zeros = [
            jnp.zeros((B * av.shape[0], *av.shape[1:]), av.dtype)
            for av in self.out_avals
        ]
        return self.sharded(*dev_args, *zeros)

    def prepare_inputs(self, x, W_in, W_out):
        jax = self.jax
        dev = {}
        if GEMM1_FP8:
            xh, xl = _split_f8(x)
            dev["xt8"] = np.concatenate(
                [_pack_xt_fp8(xh), _pack_xt_fp8(xl)], axis=1
            )
            Wt = np.ascontiguousarray(W_in.T) * np.float32(W_SCALE)
            Wh, Wl = _split_f8(Wt)
            dev["w_in_hi"] = np.concatenate([_pack_w_fp8(Wh)] * B, axis=0)
            dev["w_in_lo"] = np.concatenate([_pack_w_fp8(Wl)] * B, axis=0)
        else:
            dev["xt"] = _pack_xt_bf16(x)
            w_in_b = _pack_w_bf16(np.ascontiguousarray(W_in.T))
            dev["w_in"] = np.concatenate([w_in_b] * B, axis=0)
        if GEMM2_FP8:
            rows = np.concatenate(
                [np.arange(h * 128, (h + 1) * 128) for h in BANDED_HEADS]
            )
            Wq = np.ascontiguousarray(W_out.T[rows, :]) * np.float32(W_SCALE)
            Wh, Wl = _split_f8(Wq)
            dev["w_out_hi"] = np.concatenate([_pack_wo_fp8(Wh)] * B, axis=0)
            dev["w_out_lo"] = np.concatenate([_pack_wo_fp8(Wl)] * B, axis=0)
        else:
            w_out_b = _pack_w_bf16(np.ascontiguousarray(W_out.T))
            dev["w_out"] = np.concatenate([w_out_b] * B, axis=0)
        dev["a_tab"] = np.concatenate([self.a_tab_np] * B, axis=0)
        dev["r34t"] = _host_r34(x, W_in, W_out).reshape(B * 128, 8)
        return [jax.device_put(dev[name]) for name in self.in_names]

    def __call__(self, x, W_in, W_out):
        args = self.prepare_inputs(x, W_in, W_out)
        outs = self.run_device(args)
        outT = np.asarray(outs[self.out_names.index("outT")])
        # outT: [B*E, L] bf16 -> [B, L, E] fp32
        return np.ascontiguousarray(
            outT.reshape(B, E, L).transpose(0, 2, 1)
        ).astype(np.float32)


_CACHE = {}


def _get_runner() -> _Runner:
    if "runner" not in _CACHE:
        _CACHE["runner"] = _Runner()
    return _CACHE["runner"]


def kernel(x, W_in, W_out):
    x = np.ascontiguousarray(np.asarray(x, dtype=np.float32))
    W_in = np.ascontiguousarray(np.asarray(W_in, dtype=np.float32))
    W_out = np.ascontiguousarray(np.asarray(W_out, dtype=np.float32))
    assert x.shape == (B, L, E)
    return _get_runner()(x, W_in, W_out)


if __name__ == "__main__":
    rng = np.random.default_rng(0)
    x = rng.standard_normal((B, L, E), dtype=np.float32)
    W_in = rng.standard_normal((E, E), dtype=np.float32) * 0.05
    W_out = rng.standard_normal((E, E), dtype=np.float32) * 0.05
    y = kernel(x, W_in, W_out)
    print("out", y.shape, y.dtype, np.abs(y).mean())


# revision 42
# speedup vs baseline: 1.5398x; 1.0044x over previous
"""Trainium2 Bass kernel for nn_NewAttention (analytic Gaussian sparse attention).

Math (per batch element b):
    v        = x[b] @ W_in.T                      # [L, E]
    per head h (P=128 cols of v):
        A_h  = softmax(-(j - c_h(i))^2 / 2)       # [L, L], analytic, banded
        att_h = A_h @ v_h                         # [L, P]
    out[b]   = concat_h(att_h) @ W_out.T          # [L, E]

Sharding: data-parallel over batch, one batch element per NeuronCore (8 cores).

Device strategy (per core):
  - 'first'/'last' heads (3/4) attend to a fixed key location for every query,
    so their output contribution is a single row vector r34[e] added to every
    output row. r34 only depends on 32 rows of x, so the HOST computes it
    exactly and ships it as a per-core [128, 8] bias table. Heads 3/4 then
    vanish from the device program entirely.
  - matmul1 (v = x @ W_in.T): fp8e4 DoubleRow with a dual-plane
    error-compensated split: x = xh + xl, 64*W = Wh + Wl (same scale for all
    planes), accumulate xh@Wh + xh@Wl + xl@Wh in one PSUM group; the dropped
    xl@Wl term and plane-residuals are ~0.2% — bf16-level accuracy at half
    the PE cost. The 1/64 descale rides the PSUM->SBUF copy for free.
  - attention: att^T_h = v_h.T @ A_h^T as banded bf16 matmuls: stationary =
    v 128x128 slices, moving = analytic A^T window blocks (host-precomputed
    exact softmax weights, truncated at |key-center| <= 4). All interior tiles
    share ONE shift-invariant [128, 136] window table; boundary tiles get
    exact renormalized tables. Windows split at PSUM-bank (512 col)
    boundaries and accumulate via per-element has_written bits. Attention
    column-groups are emitted inside the phase-1 tile loop as soon as their
    v tiles exist, so their PSUM->SBUF copies hide under phase-1 matmuls.
  - matmul2 computes out^T (feature-major): stationary = W_out^T slices,
    moving = att^T q-chunks, accumulated over the 6 banded heads — also in
    dual-plane fp8 DoubleRow (att planes are split on-device: hi = 8*att via
    one copy, lo = (8*att - hi) via one scalar_tensor_tensor). The r34 bias
    and the 1/(8*64) descale ride the PSUM->SBUF copy. Output leaves the
    device as bf16 out^T; the host transposes and upcasts.
  - PE p-state: dummy matmuls on a zeroed scratch tile run during the initial
    DMA fill so the clock ramp completes before real work arrives.
"""

import sys
import numpy as np

for _p in ("/opt/trn_rl_repo",):
    if _p not in sys.path:
        sys.path.insert(0, _p)

import concourse.bass as bass
import concourse.bacc as bacc
import concourse.mybir as mybir
from concourse import tile
from concourse import bass2jax as _b2j

# ---------------- problem constants (hardcoded per contract) ----------------
B = 8
L = 2048
E = 1024
H = 8
P = 128
SIGMA = 1.0
DISP = 1
NT = L // 128           # 16 seq tiles
R = 4                   # Gaussian taps kept each side of the center
WIN = 128 + 2 * R       # 136: per-tile attention query window
DT = mybir.dt.float32
BF = mybir.dt.bfloat16
F8 = mybir.dt.float8e4

GEMM1_FP8 = True    # dual-plane fp8e4 DoubleRow for v = x @ W_in.T
GEMM2_FP8 = True    # dual-plane fp8e4 DoubleRow for out^T = W_out @ att^T
W_SCALE = 64.0          # fp8 plane scale for W_in / W_out
ATT_SCALE = 8.0         # fp8 plane scale for att^T

BANDED_HEADS = [0, 1, 2, 5, 6, 7]   # center,left,right,center,left,right
NBH = len(BANDED_HEADS)
BI_TYPE = [0, 1, 2, 0, 1, 2]        # 0=center,1=left,2=right
TYPE_DISP = [0, -DISP, +DISP]
NQ4 = L // 512                      # 4 attention PSUM column groups
# emit attention (q4 group, head pair) at the end of phase-1 tile iteration i
ATT_EMIT_AT = {
    5: (0, 0), 6: (0, 1), 7: (0, 2),
    9: (1, 0), 10: (1, 1), 11: (1, 2),
    12: (2, 0), 13: (2, 1), 14: (2, 2),
    15: (3, 0),
}


def _g(x):
    return np.exp(-(np.asarray(x, dtype=np.float64) ** 2) / (2.0 * SIGMA**2))


def _attn_tables():
    """[128, 7*136] float: interior | first(t=0..2) | last(t=0..2) A^T blocks.

    interior[k, c] = g(k + R - c)/Zinf  (shift-invariant, shared by all head
    types and tiles 1..14; the head displacement only moves the window).
    first/last blocks are exact full-softmax values at the sequence edges.
    """
    tab = np.zeros((128, 7 * WIN), dtype=np.float64)
    zinf = _g(np.arange(-64, 65)).sum()
    k = np.arange(128)
    c = np.arange(WIN)
    delta = k[:, None] + R - c[None, :]
    tab[:, 0:WIN] = np.where(np.abs(delta) <= R, _g(delta) / zinf, 0.0)

    j = np.arange(L, dtype=np.float64)
    for t in range(3):
        d = TYPE_DISP[t]
        # first block: tile 0, queries q in [0, 132 - d)
        w = 132 - d
        q = np.arange(w, dtype=np.float64)
        logits = _g(j[None, :] - (q[:, None] + d))          # [w, L]
        A = logits / logits.sum(axis=1, keepdims=True)
        tab[:, (1 + t) * WIN : (1 + t) * WIN + w] = A[:, 0:128].T
        # last block: tile 15, queries q in [1916 - d, 2048)
        w = 132 + d
        q = np.arange(L - w, L, dtype=np.float64)
        logits = _g(j[None, :] - (q[:, None] + d))
        A = logits / logits.sum(axis=1, keepdims=True)
        tab[:, (4 + t) * WIN : (4 + t) * WIN + w] = A[:, L - 128 : L].T
    return tab


def _attn_pieces():
    """pieces[t][q4] = ordered [(tile i, col within 512-psum, width, table col)]."""
    pieces = [[[] for _ in range(NQ4)] for _ in range(3)]
    for t in range(3):
        d = TYPE_DISP[t]
        for i in range(NT):
            if i == 0:
                w0, w, base = 0, 132 - d, (1 + t) * WIN
            elif i == NT - 1:
                w0, w, base = 128 * i - R - d, 132 + d, (4 + t) * WIN
            else:
                w0, w, base = 128 * i - R - d, WIN, 0
            p0 = w0
            while p0 < w0 + w:
                q4 = p0 // 512
                pend = min(w0 + w, (q4 + 1) * 512)
                pieces[t][q4].append((i, p0 - 512 * q4, pend - p0, base + p0 - w0))
                p0 = pend
    return pieces


ATT_PIECES = _attn_pieces()

# GEMM1 column chunks: banded heads only (0-2 -> [0,384), 5-7 -> [640,1024))
G1_CHUNKS = ((0, 384), (640, 384))


def _build_program(phases=3):
    nc = bacc.Bacc("TRN2", target_bir_lowering=False, debug=False, num_devices=B)

    # w_in ships only the 6 banded heads' 6144 output columns, m-half-major:
    # [mh][kc/kt][(i)][384] so each m-half is one contiguous early DMA.
    if GEMM1_FP8:
        xt8 = nc.dram_tensor("xt8", [L, 2 * E], F8, kind="ExternalInput")
        w_in_hi = nc.dram_tensor("w_in_hi", [128, 6144], F8, kind="ExternalInput")
        w_in_lo = nc.dram_tensor("w_in_lo", [128, 6144], F8, kind="ExternalInput")
    else:
        xt = nc.dram_tensor("xt", [L, E], BF, kind="ExternalInput")
        w_in = nc.dram_tensor("w_in", [128, 6144], BF, kind="ExternalInput")
    if GEMM2_FP8:
        w_out_hi = nc.dram_tensor("w_out_hi", [128, NBH * E], F8, kind="ExternalInput")
        w_out_lo = nc.dram_tensor("w_out_lo", [128, NBH * E], F8, kind="ExternalInput")
    else:
        w_out = nc.dram_tensor("w_out", [128, 8 * E], BF, kind="ExternalInput")
    a_tab = nc.dram_tensor("a_tab", [128, 7 * WIN], BF, kind="ExternalInput")
    r34t = nc.dram_tensor("r34t", [128, 8], DT, kind="ExternalInput")
    outT = nc.dram_tensor("outT", [E, L], BF, kind="ExternalOutput")

    with tile.TileContext(nc) as tc:
        with (
            tc.tile_pool(name="const", bufs=1) as cpool,
            tc.tile_pool(name="vbuf", bufs=1) as vpool,
            tc.tile_pool(name="outp", bufs=4) as outpool,
            tc.tile_pool(name="ps8", bufs=8, space="PSUM") as ps8,
        ):
            if GEMM2_FP8:
                wo_hi_sb = cpool.tile([128, NBH * E], F8, tag="wo_hi_sb")
                wo_lo_sb = cpool.tile([128, NBH * E], F8, tag="wo_lo_sb")
                att_hi = cpool.tile([128, 2 * NBH * E], F8, tag="att_hi")
                att_lo = cpool.tile([128, 2 * NBH * E], F8, tag="att_lo")
            else:
                w_out_sb = cpool.tile([128, 8 * E], BF, tag="w_out_sb")
                att_sb = cpool.tile([128, 2 * NBH * E], BF, tag="att_sb")
            a_sb = cpool.tile([128, 7 * WIN], BF, tag="a_sb")
            r34_sb = cpool.tile([128, 8], DT, tag="r34_sb")
            scratch = cpool.tile([128, 512], BF, tag="scratch")
            v_sb = vpool.tile([128, NT * E], BF, tag="v_sb")

            # ---- PE warmup: ramp the p-state during the DMA fill ----
            nc.vector.memset(scratch[:, 0:128], 0.0)
            nc.vector.memset(scratch[:, 128:512], 0.0)
            pw = ps8.tile([128, 512], DT, tag="ps")
            for _ in range(3):
                nc.tensor.matmul(
                    pw[:, 0:128], scratch[:, 0:128], scratch[:, 0:128],
                    start=True, stop=True,
                )
            for _ in range(8):
                nc.tensor.matmul(
                    pw[:], scratch[:, 0:128], scratch[:, 0:512],
                    start=True, stop=True,
                )

            # attention helpers -------------------------------------------
            copy_rr = [0]

            def emit_att_group(q4, pair=None):
                heads = list(enumerate(BANDED_HEADS))
                if pair is not None:
                    heads = heads[2 * pair : 2 * pair + 2]
                for bi, h in heads:
                    ms = ATT_PIECES[BI_TYPE[bi]][q4]
                    patt_t = ps8.tile([128, 512], DT, tag="ps")
                    patt = patt_t[:]
                    for n_, (i, col, wd, tcol) in enumerate(ms):
                        nc.tensor.matmul(
                            patt[:, col : col + wd],
                            v_sb[:, i * E + h * 128 : i * E + (h + 1) * 128],
                            a_sb[:, tcol : tcol + wd],
                            start=(n_ == 0),
                            stop=(n_ == len(ms) - 1),
                        )
                    s, qq = q4 // 2, q4 % 2
                    if GEMM2_FP8:
                        c, j = bi // 2, bi % 2
                        base = ((s * 3 + c) * 2 + j) * E + qq * 512
                        hi_dst = att_hi[:, base : base + 512]
                        lo_dst = att_lo[:, base : base + 512]
                        nc.scalar.activation(
                            hi_dst, patt,
                            mybir.ActivationFunctionType.Copy,
                            scale=ATT_SCALE,
                        )
                        nc.vector.scalar_tensor_tensor(
                            lo_dst, patt, ATT_SCALE, hi_dst,
                            mybir.AluOpType.mult, mybir.AluOpType.subtract,
                        )
                    else:
                        dst = att_sb[:, (s * NBH + bi) * E + qq * 512 :][:, :512]
                        if copy_rr[0] % 2 == 0:
                            nc.scalar.copy(dst, patt)
                        else:
                            nc.vector.tensor_copy(dst, patt)
                    copy_rr[0] += 1

            # ---- phase 1: v = x @ W_in.T (banded-head columns only) ----
            with (
                tc.tile_pool(name="w_in_p", bufs=1) as wpool,
                tc.tile_pool(name="xt_p", bufs=7) as xtpool,
            ):
                if GEMM1_FP8:
                    w_hi_sb = wpool.tile([128, 6144], F8, tag="w_hi_sb")
                    w_lo_sb = wpool.tile([128, 6144], F8, tag="w_lo_sb")

                    def load_xt(i):
                        t8 = xtpool.tile([128, 2 * E], F8, tag="xt")
                        nc.sync.dma_start(t8[:], xt8[i * 128 : (i + 1) * 128, :])
                        return t8[:, 0:E], t8[:, E : 2 * E]

                    def load_w(mh, hf):
                        c0 = mh * 3072 + hf * 1536
                        nc.sync.dma_start(
                            w_hi_sb[:, c0 : c0 + 1536],
                            w_in_hi[:, c0 : c0 + 1536],
                        )
                        nc.sync.dma_start(
                            w_lo_sb[:, c0 : c0 + 1536],
                            w_in_lo[:, c0 : c0 + 1536],
                        )
                else:
                    w_in_sb = wpool.tile([128, 6144], BF, tag="w_in_sb")

                    def load_xt(i):
                        t_ = xtpool.tile([128, E], BF, tag="xt")
                        nc.sync.dma_start(t_[:], xt[i * 128 : (i + 1) * 128, :])
                        return t_

                    def load_w(mh, hf):
                        c0 = mh * 3072 + hf * 1536
                        nc.sync.dma_start(
                            w_in_sb[:, c0 : c0 + 1536],
                            w_in[:, c0 : c0 + 1536],
                        )

                # stream in consumption order; xt0 first (w's sem lands last)
                xts = [load_xt(0)]
                load_w(0, 0)
                load_w(0, 1)
                xts.append(load_xt(1))
                load_w(1, 0)
                xts.append(load_xt(2))
                load_w(1, 1)
                xts.append(load_xt(3))
                xts.append(load_xt(4))

                pvs = {}
                vsc = (1.0 / W_SCALE) if GEMM1_FP8 else 1.0

                def g1_chunk(i, mh):
                    m0, n = G1_CHUNKS[mh]
                    pvc = ps8.tile([128, 512], DT, tag="ps")
                    pvs[(i, mh)] = pvc
                    pv = pvc
                    if GEMM1_FP8:
                        th, tl = xts[i]
                        for kc in range(4):
                            sh = th[:, kc * 256 : (kc + 1) * 256].rearrange(
                                "p (i l) -> p i l", i=2
                            )
                            sl = tl[:, kc * 256 : (kc + 1) * 256].rearrange(
                                "p (i l) -> p i l", i=2
                            )
                            wh = w_hi_sb[
                                :, (mh * 4 + kc) * 768 : (mh * 4 + kc + 1) * 768
                            ].rearrange("p (i m) -> p i m", i=2)
                            wl = w_lo_sb[
                                :, (mh * 4 + kc) * 768 : (mh * 4 + kc + 1) * 768
                            ].rearrange("p (i m) -> p i m", i=2)
                            for term, (s_, m_) in enumerate(
                                ((sh, wh), (sh, wl), (sl, wh))
                            ):
                                nc.tensor.matmul(
                                    pv[:, 0:n],
                                    s_,
                                    m_,
                                    start=(kc == 0 and term == 0),
                                    stop=(kc == 3 and term == 2),
                                    perf_mode=mybir.MatmulPerfMode.DoubleRow,
                                )
                    else:
                        xt_t = xts[i]
                        for kt in range(8):
                            nc.tensor.matmul(
                                pv[:, 0:n],
                                xt_t[:, kt * 128 : (kt + 1) * 128],
                                w_in_sb[
                                    :, (mh * 8 + kt) * 384 : (mh * 8 + kt + 1) * 384
                                ],
                                start=(kt == 0),
                                stop=(kt == 7),
                            )

                def g1_copy(i, mh):
                    m0, n = G1_CHUNKS[mh]
                    pv = pvs.pop((i, mh))
                    dst = v_sb[:, i * E + m0 : i * E + m0 + n]
                    if (i + mh) % 2 == 0:
                        nc.scalar.activation(
                            dst, pv[:, 0:n],
                            mybir.ActivationFunctionType.Copy, scale=vsc,
                        )
                    else:
                        if GEMM1_FP8:
                            nc.vector.tensor_scalar_mul(dst, pv[:, 0:n], vsc)
                        else:
                            nc.vector.tensor_copy(dst, pv[:, 0:n])

                # tiles 0/1 interleave m-halves so PE work tracks DMA arrival
                for i_, mh_ in ((0, 0), (1, 0), (0, 1), (1, 1)):
                    g1_chunk(i_, mh_)
                    g1_copy(i_, mh_)

                xts.append(load_xt(5))
                nc.sync.dma_start(a_sb[:], a_tab[:])
                nc.sync.dma_start(r34_sb[:], r34t[:])

                for i in range(2, NT):
                    if i + 4 < NT:
                        xts.append(load_xt(i + 4))
                    if GEMM2_FP8:
                        if 4 <= i < 7:
                            c = i - 4
                            nc.sync.dma_start(
                                wo_hi_sb[:, c * 2048 : (c + 1) * 2048],
                                w_out_hi[:, c * 2048 : (c + 1) * 2048],
                            )
                            nc.sync.dma_start(
                                wo_lo_sb[:, c * 2048 : (c + 1) * 2048],
                                w_out_lo[:, c * 2048 : (c + 1) * 2048],
                            )
                    else:
                        if 4 <= i < 12:
                            c = i - 4
                            nc.sync.dma_start(
                                w_out_sb[:, c * 1024 : (c + 1) * 1024],
                                w_out[:, c * 1024 : (c + 1) * 1024],
                            )
                    for mh_ in (0, 1):
                        g1_chunk(i, mh_)
                    for mh_ in (0, 1):
                        g1_copy(i, mh_)
                    if i in ATT_EMIT_AT:
                        q4_, pair_ = ATT_EMIT_AT[i]
                        emit_att_group(q4_, pair_)

            emit_att_group(3, 1)
            emit_att_group(3, 2)

            # ---- phase 3: out^T = W_out @ att^T + r34 bias ----
            osc = 1.0 / (ATT_SCALE * W_SCALE)
            nout = 0
            for qc in range(4):
                s, qq = qc // 2, qc % 2
                for et in range(8):
                    po_t = ps8.tile([128, 512], DT, tag="ps")
                    po = po_t[:]
                    # final chunk splits so the very last output DMA is tiny;
                    # sub-chunk 2 gets its own PSUM bank so its matmuls don't
                    # serialize behind sub-chunk 1's PSUM read (bank tracker)
                    last = qc == 3 and et == 7
                    subs = ((0, 384), (384, 128)) if last else ((0, 512),)
                    for si, (s0, sn) in enumerate(subs):
                        if si == 1:
                            po_t2 = ps8.tile([128, 512], DT, tag="ps")
                            po = po_t2[:]
                        p0 = 0 if si == 1 else s0
                        if GEMM2_FP8:
                            nmm = 0
                            for c in range(3):
                                rh = att_hi[
                                    :, (s * 3 + c) * 2048 :][:, :2048].rearrange(
                                    "p (j q) -> p j q", j=2
                                )[:, :, qq * 512 + s0 : qq * 512 + s0 + sn]
                                rl = att_lo[
                                    :, (s * 3 + c) * 2048 :][:, :2048].rearrange(
                                    "p (j q) -> p j q", j=2
                                )[:, :, qq * 512 + s0 : qq * 512 + s0 + sn]
                                lh = wo_hi_sb[
                                    :, c * 2048 : (c + 1) * 2048
                                ].rearrange("p (j e) -> p j e", j=2)[
                                    :, :, et * 128 : (et + 1) * 128
                                ]
                                ll = wo_lo_sb[
                                    :, c * 2048 : (c + 1) * 2048
                                ].rearrange("p (j e) -> p j e", j=2)[
                                    :, :, et * 128 : (et + 1) * 128
                                ]
                                for s_, m_ in ((lh, rh), (ll, rh), (lh, rl)):
                                    nc.tensor.matmul(
                                        po[:, p0 : p0 + sn], s_, m_,
                                        start=(nmm == 0),
                                        stop=(nmm == 8),
                                        perf_mode=mybir.MatmulPerfMode.DoubleRow,
                                    )
                                    nmm += 1
                        else:
                            for bi, h in enumerate(BANDED_HEADS):
                                nc.tensor.matmul(
                                    po[:, p0 : p0 + sn],
                                    w_out_sb[
                                        :, h * E + et * 128 : h * E + (et + 1) * 128
                                    ],
                                    att_sb[
                                        :, (s * NBH + bi) * E + qq * 512 + s0 :
                                    ][:, :sn],
                                    start=(bi == 0),
                                    stop=(bi == NBH - 1),
                                )
                        ot = outpool.tile([128, 512], BF, tag="ot")
                        bias = r34_sb[:, et : et + 1]
                        use_act = (nout % 2 == 0) if not last else (si == 0)
                        if GEMM2_FP8:
                            if use_act:
                                nc.scalar.activation(
                                    ot[:, 0:sn], po[:, p0 : p0 + sn],
                                    mybir.ActivationFunctionType.Identity,
                                    bias=bias, scale=osc,
                                )
                            else:
                                nc.vector.tensor_scalar(
                                    ot[:, 0:sn], po[:, p0 : p0 + sn], osc, bias,
                                    mybir.AluOpType.mult, mybir.AluOpType.add,
                                )
                        else:
                            if use_act:
                                nc.scalar.add(ot[:, 0:sn], po[:, p0 : p0 + sn], bias)
                            else:
                                nc.vector.tensor_scalar_add(
                                    ot[:, 0:sn], po[:, p0 : p0 + sn], bias
                                )
                        nout += 1
                        dma_eng = nc.sync
                        if last and si == 0:
                            dma_eng = nc.scalar
                        dma_eng.dma_start(
                            outT[
                                et * 128 : (et + 1) * 128,
                                qc * 512 + s0 : qc * 512 + s0 + sn,
                            ],
                            ot[:, 0:sn],
                        )

    nc.compile()
    return nc


# ------------------------- host-side preparation ---------------------------

_NPBF = mybir.dt.np(BF)
_NPF8 = mybir.dt.np(F8)
# banded-head output columns of W_in^T, m-half-major (0:384 then 640:1024)
_MCOLS = np.concatenate([np.arange(0, 384), np.arange(640, 1024)])


def _host_wf_wl():
    """Exact 'first'/'last' head weight vectors over their 16-key support."""
    j = np.arange(L, dtype=np.float64)
    zf = _g(j - 0.0).sum()
    zl = _g(j - (L - 1.0)).sum()
    wf = _g(np.arange(16)) / zf
    wl = _g(np.arange(L - 16, L) - (L - 1.0)) / zl
    return wf, wl


def _host_r34(x, W_in, W_out):
    """[B, 128, 8] fp32: per-core output bias rows from the 'first'/'last'
    heads, computed exactly on the host (r34t[p, et] = r34[et*128 + p])."""
    wf, wl = _host_wf_wl()
    x64 = x.astype(np.float64)
    s3 = np.einsum("k,bke->be", wf, x64[:, 0:16, :])        # [B, E]
    s4 = np.einsum("k,bke->be", wl, x64[:, L - 16 : L, :])
    W_in64 = W_in.astype(np.float64)
    W_out64 = W_out.astype(np.float64)
    u3 = s3 @ W_in64.T[:, 384:512]                          # [B, 128]
    u4 = s4 @ W_in64.T[:, 512:640]
    r34 = u3 @ W_out64.T[384:512, :] + u4 @ W_out64.T[512:640, :]  # [B, E]
    return np.ascontiguousarray(
        r34.reshape(B, 8, 128).transpose(0, 2, 1)
    ).astype(np.float32)


def _pack_xt_bf16(x):
    # xt[b, i*128 + p, kt*128 + l] = x[b, i*128 + l, kt*128 + p]
    t = x.reshape(B, NT, 128, 8, 128).transpose(0, 1, 4, 3, 2)
    return np.ascontiguousarray(t).reshape(B * L, E).astype(_NPBF)


def _pack_xt_fp8(xq):
    # xt[b, i*128 + p, kc*256 + ipl*128 + l] = xq[b, i*128 + l, kc*256 + ipl*128 + p]
    t = xq.reshape(B, NT, 128, 4, 2, 128).transpose(0, 1, 5, 3, 4, 2)
    return np.ascontiguousarray(t).reshape(B * L, E)


def _pack_w_bf16(Wt):
    # w[p, (mh*8 + kt)*384 + m] = W.T[kt*128 + p, mcol(mh, m)]
    t = Wt.reshape(8, 128, E)[:, :, _MCOLS]          # [kt, p, mh*384+m]
    t = t.reshape(8, 128, 2, 384).transpose(1, 2, 0, 3)
    return np.ascontiguousarray(t).reshape(128, 6144).astype(_NPBF)


def _pack_w_fp8(Wq):
    # w[p, ((mh*4 + kc)*2 + ipl)*384 + m] = Wq[kc*256 + ipl*128 + p, mcol(mh, m)]
    t = Wq.reshape(4, 2, 128, E)[:, :, :, _MCOLS]    # [kc, ipl, p, mh*384+m]
    t = t.reshape(4, 2, 128, 2, 384).transpose(2, 3, 0, 1, 4)
    return np.ascontiguousarray(t).reshape(128, 6144)


def _pack_wo_fp8(Wq):
    # Wq: [NBH*128, E] rows = banded-head-major features (bi, p).
    # wo[p, c*2048 + j*1024 + e] = Wq[(c*2 + j)*128 + p, e]
    t = Wq.reshape(3, 2, 128, E).transpose(2, 0, 1, 3)
    return np.ascontiguousarray(t).reshape(128, NBH * E)


def _split_f8(a):
    hi = a.astype(_NPF8)
    lo = (a - hi.astype(np.float32)).astype(_NPF8)
    return hi, lo


class _Runner:
    """Builds the Bass program once and caches a jitted shard_map executable
    (one batch element per NeuronCore)."""

    def __init__(self):
        import jax
        from jax.sharding import Mesh, PartitionSpec
        from jax.experimental.shard_map import shard_map

        self.jax = jax
        _b2j.install_neuronx_cc_hook()
        nc = _build_program()
        self.nc = nc
        self.a_tab_np = _attn_tables().astype(_NPBF)

        partition_name = (
            nc.partition_id_tensor.name if nc.partition_id_tensor else None
        )
        in_names = []
        out_names = []
        out_avals = []
        for alloc in nc.m.functions[0].allocations:
            if not isinstance(alloc, mybir.MemoryLocationSet):
                continue
            name = alloc.memorylocations[0].name
            if alloc.kind == "ExternalInput":
                if name != partition_name:
                    in_names.append(name)
            elif alloc.kind == "ExternalOutput":
                out_names.append(name)
                out_avals.append(
                    jax.core.ShapedArray(
                        tuple(alloc.tensor_shape), mybir.dt.np(alloc.dtype)
                    )
                )
        self.in_names = in_names
        self.out_names = out_names
        self.out_avals = out_avals
        n_params = len(in_names)
        n_outs = len(out_names)
        all_names = tuple(in_names) + tuple(out_names)
        if partition_name is not None:
            all_names = all_names + (partition_name,)

        def _body(*args):
            operands = list(args)
            if partition_name is not None:
                operands.append(_b2j.partition_id_tensor())
            outs = _b2j._bass_exec_p.bind(
                *operands,
                out_avals=tuple(out_avals),
                in_names=all_names,
                out_names=tuple(out_names),
                lowering_input_output_aliases=(),
                sim_require_finite=True,
                sim_require_nnan=True,
                nc=nc,
            )
            return tuple(outs)

        devices = jax.devices()[:B]
        assert len(devices) == B
        self.mesh = Mesh(np.asarray(devices), ("core",))
        in_specs = (PartitionSpec("core"),) * (n_params + n_outs)
        out_specs = (PartitionSpec("core"),) * n_outs
        self.sharded = jax.jit(
            shard_map(
                _body,
                mesh=self.mesh,
                in_specs=in_specs,
                out_specs=out_specs,
                check_rep=False,
            ),
            donate_argnums=tuple(range(n_params, n_params + n_outs)),
            keep_unused=True,
        )

    def run_device(self, dev_args):
        jnp = self.jax.numpy
        zeros = [
            jnp.zeros((B * av.shape[0], *av.shape[1:]), av.dtype)
            for av in self.out_avals
        ]
        return self.sharded(*dev_args, *zeros)

    def prepare_inputs(self, x, W_in, W_out):
        jax = self.jax
        dev = {}
        if GEMM1_FP8:
            xh, xl = _split_f8(x)
            dev["xt8"] = np.concatenate(
                [_pack_xt_fp8(xh), _pack_xt_fp8(xl)], axis=1
            )
            Wt = np.ascontiguousarray(W_in.T) * np.float32(W_SCALE)
            Wh, Wl = _split_f8(Wt)
            dev["w_in_hi"] = np.concatenate([_pack_w_fp8(Wh)] * B, axis=0)
            dev["w_in_lo"] = np.concatenate([_pack_w_fp8(Wl)] * B, axis=0)
        else:
            dev["xt"] = _pack_xt_bf16(x)
            w_in_b = _pack_w_bf16(np.ascontiguousarray(W_in.T))
            dev["w_in"] = np.concatenate([w_in_b] * B, axis=0)
        if GEMM2_FP8:
            rows = np.concatenate(
                [np.arange(h * 128, (h + 1) * 128) for h in BANDED_HEADS]
            )
            Wq = np.ascontiguousarray(W_out.T[rows, :]) * np.float32(W_SCALE)
            Wh, Wl = _split_f8(Wq)
            dev["w_out_hi"] = np.concatenate([_pack_wo_fp8(Wh)] * B, axis=0)
            dev["w_out_lo"] = np.concatenate([_pack_wo_fp8(Wl)] * B, axis=0)
        else:
            w_out_b = _pack_w_bf16(np.ascontiguousarray(W_out.T))
            dev["w_out"] = np.concatenate([w_out_b] * B, axis=0)
        dev["a_tab"] = np.concatenate([self.a_tab_np] * B, axis=0)
        dev["r34t"] = _host_r34(x, W_in, W_out).reshape(B * 128, 8)
        return [jax.device_put(dev[name]) for name in self.in_names]

    def __call__(self, x, W_in, W_out):
        args = self.prepare_inputs(x, W_in, W_out)
        outs = self.run_device(args)
        outT = np.asarray(outs[self.out_names.index("outT")])
        # outT: [B*E, L] bf16 -> [B, L, E] fp32
        return np.ascontiguousarray(
            outT.reshape(B, E, L).transpose(0, 2, 1)
        ).astype(np.float32)


_CACHE = {}


def _get_runner() -> _Runner:
    if "runner" not in _CACHE:
        _CACHE["runner"] = _Runner()
    return _CACHE["runner"]


def kernel(x, W_in, W_out):
    x = np.ascontiguousarray(np.asarray(x, dtype=np.float32))
    W_in = np.ascontiguousarray(np.asarray(W_in, dtype=np.float32))
    W_out = np.ascontiguousarray(np.asarray(W_out, dtype=np.float32))
    assert x.shape == (B, L, E)
    return _get_runner()(x, W_in, W_out)


if __name__ == "__main__":
    rng = np.random.default_rng(0)
    x = rng.standard_normal((B, L, E), dtype=np.float32)
    W_in = rng.standard_normal((E, E), dtype=np.float32) * 0.05
    W_out = rng.standard_normal((E, E), dtype=np.float32) * 0.05
    y = kernel(x, W_in, W_out)
    print("out", y.shape, y.dtype, np.abs(y).mean())


# revision 48
# speedup vs baseline: 1.5413x; 1.0010x over previous
"""Trainium2 Bass kernel for nn_NewAttention (analytic Gaussian sparse attention).

Math (per batch element b):
    v        = x[b] @ W_in.T                      # [L, E]
    per head h (P=128 cols of v):
        A_h  = softmax(-(j - c_h(i))^2 / 2)       # [L, L], analytic, banded
        att_h = A_h @ v_h                         # [L, P]
    out[b]   = concat_h(att_h) @ W_out.T          # [L, E]

Sharding: data-parallel over batch, one batch element per NeuronCore (8 cores).

Device strategy (per core):
  - 'first'/'last' heads (3/4) attend to a fixed key location for every query,
    so their output contribution is a single row vector r34[e] added to every
    output row. r34 only depends on 32 rows of x, so the HOST computes it
    exactly and ships it as a per-core [128, 8] bias table. Heads 3/4 then
    vanish from the device program entirely.
  - matmul1 (v = x @ W_in.T): fp8e4 DoubleRow with a dual-plane
    error-compensated split: x = xh + xl, 64*W = Wh + Wl (same scale for all
    planes), accumulate xh@Wh + xh@Wl + xl@Wh in one PSUM group; the dropped
    xl@Wl term and plane-residuals are ~0.2% — bf16-level accuracy at half
    the PE cost. The 1/64 descale rides the PSUM->SBUF copy for free.
  - attention: att^T_h = v_h.T @ A_h^T as banded bf16 matmuls: stationary =
    v 128x128 slices, moving = analytic A^T window blocks (host-precomputed
    exact softmax weights, truncated at |key-center| <= 4). All interior tiles
    share ONE shift-invariant [128, 136] window table; boundary tiles get
    exact renormalized tables. Windows split at PSUM-bank (512 col)
    boundaries and accumulate via per-element has_written bits. Attention
    column-groups are emitted inside the phase-1 tile loop as soon as their
    v tiles exist, so their PSUM->SBUF copies hide under phase-1 matmuls.
  - matmul2 computes out^T (feature-major): stationary = W_out^T slices,
    moving = att^T q-chunks, accumulated over the 6 banded heads — also in
    dual-plane fp8 DoubleRow (att planes are split on-device: hi = 8*att via
    one copy, lo = (8*att - hi) via one scalar_tensor_tensor). The r34 bias
    and the 1/(8*64) descale ride the PSUM->SBUF copy. Output leaves the
    device as bf16 out^T; the host transposes and upcasts.
  - PE p-state: dummy matmuls on a zeroed scratch tile run during the initial
    DMA fill so the clock ramp completes before real work arrives.
"""

import sys
import numpy as np

for _p in ("/opt/trn_rl_repo",):
    if _p not in sys.path:
        sys.path.insert(0, _p)

import concourse.bass as bass
import concourse.bacc as bacc
import concourse.mybir as mybir
from concourse import tile
from concourse import bass2jax as _b2j

# ---------------- problem constants (hardcoded per contract) ----------------
B = 8
L = 2048
E = 1024
H = 8
P = 128
SIGMA = 1.0
DISP = 1
NT = L // 128           # 16 seq tiles
R = 4                   # Gaussian taps kept each side of the center
WIN = 128 + 2 * R       # 136: per-tile attention query window
DT = mybir.dt.float32
BF = mybir.dt.bfloat16
F8 = mybir.dt.float8e4

GEMM1_FP8 = True    # dual-plane fp8e4 DoubleRow for v = x @ W_in.T
GEMM2_FP8 = True    # dual-plane fp8e4 DoubleRow for out^T = W_out @ att^T
W_SCALE = 64.0          # fp8 plane scale for W_in / W_out
ATT_SCALE = 8.0         # fp8 plane scale for att^T

BANDED_HEADS = [0, 1, 2, 5, 6, 7]   # center,left,right,center,left,right
NBH = len(BANDED_HEADS)
BI_TYPE = [0, 1, 2, 0, 1, 2]        # 0=center,1=left,2=right
TYPE_DISP = [0, -DISP, +DISP]
NQ4 = L // 512                      # 4 attention PSUM column groups
# emit attention (q4 group, head pair) at the end of phase-1 tile iteration i
ATT_EMIT_AT = {
    5: (0, 0), 6: (0, 1), 7: (0, 2),
    9: (1, 0), 10: (1, 1), 11: (1, 2),
    12: (2, 0), 13: (2, 1), 14: (2, 2),
    15: (3, 0),
}


def _g(x):
    return np.exp(-(np.asarray(x, dtype=np.float64) ** 2) / (2.0 * SIGMA**2))


def _attn_tables():
    """[128, 7*136] float: interior | first(t=0..2) | last(t=0..2) A^T blocks.

    interior[k, c] = g(k + R - c)/Zinf  (shift-invariant, shared by all head
    types and tiles 1..14; the head displacement only moves the window).
    first/last blocks are exact full-softmax values at the sequence edges.
    """
    tab = np.zeros((128, 7 * WIN), dtype=np.float64)
    zinf = _g(np.arange(-64, 65)).sum()
    k = np.arange(128)
    c = np.arange(WIN)
    delta = k[:, None] + R - c[None, :]
    tab[:, 0:WIN] = np.where(np.abs(delta) <= R, _g(delta) / zinf, 0.0)

    j = np.arange(L, dtype=np.float64)
    for t in range(3):
        d = TYPE_DISP[t]
        # first block: tile 0, queries q in [0, 132 - d)
        w = 132 - d
        q = np.arange(w, dtype=np.float64)
        logits = _g(j[None, :] - (q[:, None] + d))          # [w, L]
        A = logits / logits.sum(axis=1, keepdims=True)
        tab[:, (1 + t) * WIN : (1 + t) * WIN + w] = A[:, 0:128].T
        # last block: tile 15, queries q in [1916 - d, 2048)
        w = 132 + d
        q = np.arange(L - w, L, dtype=np.float64)
        logits = _g(j[None, :] - (q[:, None] + d))
        A = logits / logits.sum(axis=1, keepdims=True)
        tab[:, (4 + t) * WIN : (4 + t) * WIN + w] = A[:, L - 128 : L].T
    return tab


def _attn_pieces():
    """pieces[t][q4] = ordered [(tile i, col within 512-psum, width, table col)]."""
    pieces = [[[] for _ in range(NQ4)] for _ in range(3)]
    for t in range(3):
        d = TYPE_DISP[t]
        for i in range(NT):
            if i == 0:
                w0, w, base = 0, 132 - d, (1 + t) * WIN
            elif i == NT - 1:
                w0, w, base = 128 * i - R - d, 132 + d, (4 + t) * WIN
            else:
                w0, w, base = 128 * i - R - d, WIN, 0
            p0 = w0
            while p0 < w0 + w:
                q4 = p0 // 512
                pend = min(w0 + w, (q4 + 1) * 512)
                pieces[t][q4].append((i, p0 - 512 * q4, pend - p0, base + p0 - w0))
                p0 = pend
    return pieces


ATT_PIECES = _attn_pieces()

# GEMM1 column chunks: banded heads only (0-2 -> [0,384), 5-7 -> [640,1024))
G1_CHUNKS = ((0, 384), (640, 384))


def _build_program(phases=3):
    nc = bacc.Bacc("TRN2", target_bir_lowering=False, debug=False, num_devices=B)

    # w_in ships only the 6 banded heads' 6144 output columns, m-half-major:
    # [mh][kc/kt][(i)][384] so each m-half is one contiguous early DMA.
    if GEMM1_FP8:
        xt8 = nc.dram_tensor("xt8", [L, 2 * E], F8, kind="ExternalInput")
        w_in_hi = nc.dram_tensor("w_in_hi", [128, 6144], F8, kind="ExternalInput")
        w_in_lo = nc.dram_tensor("w_in_lo", [128, 6144], F8, kind="ExternalInput")
    else:
        xt = nc.dram_tensor("xt", [L, E], BF, kind="ExternalInput")
        w_in = nc.dram_tensor("w_in", [128, 6144], BF, kind="ExternalInput")
    if GEMM2_FP8:
        w_out_hi = nc.dram_tensor("w_out_hi", [128, NBH * E], F8, kind="ExternalInput")
        w_out_lo = nc.dram_tensor("w_out_lo", [128, NBH * E], F8, kind="ExternalInput")
    else:
        w_out = nc.dram_tensor("w_out", [128, 8 * E], BF, kind="ExternalInput")
    a_tab = nc.dram_tensor("a_tab", [128, 7 * WIN], BF, kind="ExternalInput")
    r34t = nc.dram_tensor("r34t", [128, 8], DT, kind="ExternalInput")
    outT = nc.dram_tensor("outT", [E, L], BF, kind="ExternalOutput")

    with tile.TileContext(nc) as tc:
        with (
            tc.tile_pool(name="const", bufs=1) as cpool,
            tc.tile_pool(name="vbuf", bufs=1) as vpool,
            tc.tile_pool(name="outp", bufs=4) as outpool,
            tc.tile_pool(name="ps8", bufs=8, space="PSUM") as ps8,
        ):
            if GEMM2_FP8:
                wo_hi_sb = cpool.tile([128, NBH * E], F8, tag="wo_hi_sb")
                wo_lo_sb = cpool.tile([128, NBH * E], F8, tag="wo_lo_sb")
                att_hi = cpool.tile([128, 2 * NBH * E], F8, tag="att_hi")
                att_lo = cpool.tile([128, 2 * NBH * E], F8, tag="att_lo")
            else:
                w_out_sb = cpool.tile([128, 8 * E], BF, tag="w_out_sb")
                att_sb = cpool.tile([128, 2 * NBH * E], BF, tag="att_sb")
            a_sb = cpool.tile([128, 7 * WIN], BF, tag="a_sb")
            r34_sb = cpool.tile([128, 8], DT, tag="r34_sb")
            scratch = cpool.tile([128, 512], BF, tag="scratch")
            v_sb = vpool.tile([128, NT * E], BF, tag="v_sb")

            # ---- PE warmup: ramp the p-state during the DMA fill.
            # One memset + uniform N=128 matmuls: a single dependency so the
            # PE never micro-gaps mid-warmup (any gap resets the ramp clock).
            nc.vector.memset(scratch[:, 0:128], 0.0)
            pw = ps8.tile([128, 512], DT, tag="ps")
            for _ in range(28):
                nc.tensor.matmul(
                    pw[:, 0:128], scratch[:, 0:128], scratch[:, 0:128],
                    start=True, stop=True,
                )

            # attention helpers -------------------------------------------
            copy_rr = [0]

            def emit_att_group(q4, pair=None):
                heads = list(enumerate(BANDED_HEADS))
                if pair is not None:
                    heads = heads[2 * pair : 2 * pair + 2]
                for bi, h in heads:
                    ms = ATT_PIECES[BI_TYPE[bi]][q4]
                    patt_t = ps8.tile([128, 512], DT, tag="ps")
                    patt = patt_t[:]
                    for n_, (i, col, wd, tcol) in enumerate(ms):
                        nc.tensor.matmul(
                            patt[:, col : col + wd],
                            v_sb[:, i * E + h * 128 : i * E + (h + 1) * 128],
                            a_sb[:, tcol : tcol + wd],
                            start=(n_ == 0),
                            stop=(n_ == len(ms) - 1),
                        )
                    s, qq = q4 // 2, q4 % 2
                    if GEMM2_FP8:
                        c, j = bi // 2, bi % 2
                        base = ((s * 3 + c) * 2 + j) * E + qq * 512
                        hi_dst = att_hi[:, base : base + 512]
                        lo_dst = att_lo[:, base : base + 512]
                        nc.scalar.activation(
                            hi_dst, patt,
                            mybir.ActivationFunctionType.Copy,
                            scale=ATT_SCALE,
                        )
                        nc.vector.scalar_tensor_tensor(
                            lo_dst, patt, ATT_SCALE, hi_dst,
                            mybir.AluOpType.mult, mybir.AluOpType.subtract,
                        )
                    else:
                        dst = att_sb[:, (s * NBH + bi) * E + qq * 512 :][:, :512]
                        if copy_rr[0] % 2 == 0:
                            nc.scalar.copy(dst, patt)
                        else:
                            nc.vector.tensor_copy(dst, patt)
                    copy_rr[0] += 1

            # ---- phase 1: v = x @ W_in.T (banded-head columns only) ----
            with (
                tc.tile_pool(name="w_in_p", bufs=1) as wpool,
                tc.tile_pool(name="xt_p", bufs=7) as xtpool,
            ):
                if GEMM1_FP8:
                    w_hi_sb = wpool.tile([128, 6144], F8, tag="w_hi_sb")
                    w_lo_sb = wpool.tile([128, 6144], F8, tag="w_lo_sb")

                    def load_xt(i):
                        t8 = xtpool.tile([128, 2 * E], F8, tag="xt")
                        nc.sync.dma_start(t8[:], xt8[i * 128 : (i + 1) * 128, :])
                        return t8[:, 0:E], t8[:, E : 2 * E]

                    def load_w(mh, hf):
                        c0 = mh * 3072 + hf * 1536
                        nc.sync.dma_start(
                            w_hi_sb[:, c0 : c0 + 1536],
                            w_in_hi[:, c0 : c0 + 1536],
                        )
                        nc.sync.dma_start(
                            w_lo_sb[:, c0 : c0 + 1536],
                            w_in_lo[:, c0 : c0 + 1536],
                        )
                else:
                    w_in_sb = wpool.tile([128, 6144], BF, tag="w_in_sb")

                    def load_xt(i):
                        t_ = xtpool.tile([128, E], BF, tag="xt")
                        nc.sync.dma_start(t_[:], xt[i * 128 : (i + 1) * 128, :])
                        return t_

                    def load_w(mh, hf):
                        c0 = mh * 3072 + hf * 1536
                        nc.sync.dma_start(
                            w_in_sb[:, c0 : c0 + 1536],
                            w_in[:, c0 : c0 + 1536],
                        )

                # stream in consumption order; xt0 first (w's sem lands last)
                xts = [load_xt(0)]
                load_w(0, 0)
                load_w(0, 1)
                xts.append(load_xt(1))
                load_w(1, 0)
                xts.append(load_xt(2))
                load_w(1, 1)
                xts.append(load_xt(3))
                xts.append(load_xt(4))

                pvs = {}
                vsc = (1.0 / W_SCALE) if GEMM1_FP8 else 1.0

                def g1_chunk(i, mh):
                    m0, n = G1_CHUNKS[mh]
                    pvc = ps8.tile([128, 512], DT, tag="ps")
                    pvs[(i, mh)] = pvc
                    pv = pvc
                    if GEMM1_FP8:
                        th, tl = xts[i]
                        for kc in range(4):
                            sh = th[:, kc * 256 : (kc + 1) * 256].rearrange(
                                "p (i l) -> p i l", i=2
                            )
                            sl = tl[:, kc * 256 : (kc + 1) * 256].rearrange(
                                "p (i l) -> p i l", i=2
                            )
                            wh = w_hi_sb[
                                :, (mh * 4 + kc) * 768 : (mh * 4 + kc + 1) * 768
                            ].rearrange("p (i m) -> p i m", i=2)
                            wl = w_lo_sb[
                                :, (mh * 4 + kc) * 768 : (mh * 4 + kc + 1) * 768
                            ].rearrange("p (i m) -> p i m", i=2)
                            for term, (s_, m_) in enumerate(
                                ((sh, wh), (sl, wh), (sh, wl))
                            ):
                                nc.tensor.matmul(
                                    pv[:, 0:n],
                                    s_,
                                    m_,
                                    start=(kc == 0 and term == 0),
                                    stop=(kc == 3 and term == 2),
                                    perf_mode=mybir.MatmulPerfMode.DoubleRow,
                                )
                    else:
                        xt_t = xts[i]
                        for kt in range(8):
                            nc.tensor.matmul(
                                pv[:, 0:n],
                                xt_t[:, kt * 128 : (kt + 1) * 128],
                                w_in_sb[
                                    :, (mh * 8 + kt) * 384 : (mh * 8 + kt + 1) * 384
                                ],
                                start=(kt == 0),
                                stop=(kt == 7),
                            )

                def g1_copy(i, mh):
                    m0, n = G1_CHUNKS[mh]
                    pv = pvs.pop((i, mh))
                    dst = v_sb[:, i * E + m0 : i * E + m0 + n]
                    if (i + mh) % 2 == 0:
                        nc.scalar.activation(
                            dst, pv[:, 0:n],
                            mybir.ActivationFunctionType.Copy, scale=vsc,
                        )
                    else:
                        if GEMM1_FP8:
                            nc.vector.tensor_scalar_mul(dst, pv[:, 0:n], vsc)
                        else:
                            nc.vector.tensor_copy(dst, pv[:, 0:n])

                # tiles 0/1 interleave m-halves so PE work tracks DMA arrival
                for i_, mh_ in ((0, 0), (1, 0), (0, 1), (1, 1)):
                    g1_chunk(i_, mh_)
                    g1_copy(i_, mh_)

                xts.append(load_xt(5))
                nc.sync.dma_start(a_sb[:], a_tab[:])
                nc.sync.dma_start(r34_sb[:], r34t[:])

                for i in range(2, NT):
                    if i + 4 < NT:
                        xts.append(load_xt(i + 4))
                    if GEMM2_FP8:
                        if 4 <= i < 7:
                            c = i - 4
                            nc.sync.dma_start(
                                wo_hi_sb[:, c * 2048 : (c + 1) * 2048],
                                w_out_hi[:, c * 2048 : (c + 1) * 2048],
                            )
                            nc.sync.dma_start(
                                wo_lo_sb[:, c * 2048 : (c + 1) * 2048],
                                w_out_lo[:, c * 2048 : (c + 1) * 2048],
                            )
                    else:
                        if 4 <= i < 12:
                            c = i - 4
                            nc.sync.dma_start(
                                w_out_sb[:, c * 1024 : (c + 1) * 1024],
                                w_out[:, c * 1024 : (c + 1) * 1024],
                            )
                    for mh_ in (0, 1):
                        g1_chunk(i, mh_)
                    for mh_ in (0, 1):
                        g1_copy(i, mh_)
                    if i in ATT_EMIT_AT:
                        q4_, pair_ = ATT_EMIT_AT[i]
                        emit_att_group(q4_, pair_)

            emit_att_group(3, 1)
            emit_att_group(3, 2)

            # ---- phase 3: out^T = W_out @ att^T + r34 bias ----
            osc = 1.0 / (ATT_SCALE * W_SCALE)
            nout = 0
            for qc in range(4):
                s, qq = qc // 2, qc % 2
                for et in range(8):
                    po_t = ps8.tile([128, 512], DT, tag="ps")
                    po = po_t[:]
                    # final chunk splits so the very last output DMA is tiny;
                    # sub-chunk 2 gets its own PSUM bank so its matmuls don't
                    # serialize behind sub-chunk 1's PSUM read (bank tracker)
                    last = qc == 3 and et == 7
                    subs = ((0, 384), (384, 128)) if last else ((0, 512),)
                    for si, (s0, sn) in enumerate(subs):
                        if si == 1:
                            po_t2 = ps8.tile([128, 512], DT, tag="ps")
                            po = po_t2[:]
                        p0 = 0 if si == 1 else s0
                        if GEMM2_FP8:
                            nmm = 0
                            for c in range(3):
                                rh = att_hi[
                                    :, (s * 3 + c) * 2048 :][:, :2048].rearrange(
                                    "p (j q) -> p j q", j=2
                                )[:, :, qq * 512 + s0 : qq * 512 + s0 + sn]
                                rl = att_lo[
                                    :, (s * 3 + c) * 2048 :][:, :2048].rearrange(
                                    "p (j q) -> p j q", j=2
                                )[:, :, qq * 512 + s0 : qq * 512 + s0 + sn]
                                lh = wo_hi_sb[
                                    :, c * 2048 : (c + 1) * 2048
                                ].rearrange("p (j e) -> p j e", j=2)[
                                    :, :, et * 128 : (et + 1) * 128
                                ]
                                ll = wo_lo_sb[
                                    :, c * 2048 : (c + 1) * 2048
                                ].rearrange("p (j e) -> p j e", j=2)[
                                    :, :, et * 128 : (et + 1) * 128
                                ]
                                for s_, m_ in ((lh, rh), (ll, rh), (lh, rl)):
                                    nc.tensor.matmul(
                                        po[:, p0 : p0 + sn], s_, m_,
                                        start=(nmm == 0),
                                        stop=(nmm == 8),
                                        perf_mode=mybir.MatmulPerfMode.DoubleRow,
                                    )
                                    nmm += 1
                        else:
                            for bi, h in enumerate(BANDED_HEADS):
                                nc.tensor.matmul(
                                    po[:, p0 : p0 + sn],
                                    w_out_sb[
                                        :, h * E + et * 128 : h * E + (et + 1) * 128
                                    ],
                                    att_sb[
                                        :, (s * NBH + bi) * E + qq * 512 + s0 :
                                    ][:, :sn],
                                    start=(bi == 0),
                                    stop=(bi == NBH - 1),
                                )
                        ot = outpool.tile([128, 512], BF, tag="ot")
                        bias = r34_sb[:, et : et + 1]
                        use_act = (nout % 2 == 0) if not last else True
                        if GEMM2_FP8:
                            if use_act:
                                nc.scalar.activation(
                                    ot[:, 0:sn], po[:, p0 : p0 + sn],
                                    mybir.ActivationFunctionType.Identity,
                                    bias=bias, scale=osc,
                                )
                            else:
                                nc.vector.tensor_scalar(
                                    ot[:, 0:sn], po[:, p0 : p0 + sn], osc, bias,
                                    mybir.AluOpType.mult, mybir.AluOpType.add,
                                )
                        else:
                            if use_act:
                                nc.scalar.add(ot[:, 0:sn], po[:, p0 : p0 + sn], bias)
                            else:
                                nc.vector.tensor_scalar_add(
                                    ot[:, 0:sn], po[:, p0 : p0 + sn], bias
                                )
                        nout += 1
                        dma_eng = nc.sync
                        if last and si == 0:
                            dma_eng = nc.gpsimd
                        dma_eng.dma_start(
                            outT[
                                et * 128 : (et + 1) * 128,
                                qc * 512 + s0 : qc * 512 + s0 + sn,
                            ],
                            ot[:, 0:sn],
                        )

    nc.compile()
    return nc


# ------------------------- host-side preparation ---------------------------

_NPBF = mybir.dt.np(BF)
_NPF8 = mybir.dt.np(F8)
# banded-head output columns of W_in^T, m-half-major (0:384 then 640:1024)
_MCOLS = np.concatenate([np.arange(0, 384), np.arange(640, 1024)])


def _host_wf_wl():
    """Exact 'first'/'last' head weight vectors over their 16-key support."""
    j = np.arange(L, dtype=np.float64)
    zf = _g(j - 0.0).sum()
    zl = _g(j - (L - 1.0)).sum()
    wf = _g(np.arange(16)) / zf
    wl = _g(np.arange(L - 16, L) - (L - 1.0)) / zl
    return wf, wl


def _host_r34(x, W_in, W_out):
    """[B, 128, 8] fp32: per-core output bias rows from the 'first'/'last'
    heads, computed exactly on the host (r34t[p, et] = r34[et*128 + p])."""
    wf, wl = _host_wf_wl()
    x64 = x.astype(np.float64)
    s3 = np.einsum("k,bke->be", wf, x64[:, 0:16, :])        # [B, E]
    s4 = np.einsum("k,bke->be", wl, x64[:, L - 16 : L, :])
    W_in64 = W_in.astype(np.float64)
    W_out64 = W_out.astype(np.float64)
    u3 = s3 @ W_in64.T[:, 384:512]                          # [B, 128]
    u4 = s4 @ W_in64.T[:, 512:640]
    r34 = u3 @ W_out64.T[384:512, :] + u4 @ W_out64.T[512:640, :]  # [B, E]
    return np.ascontiguousarray(
        r34.reshape(B, 8, 128).transpose(0, 2, 1)
    ).astype(np.float32)


def _pack_xt_bf16(x):
    # xt[b, i*128 + p, kt*128 + l] = x[b, i*128 + l, kt*128 + p]
    t = x.reshape(B, NT, 128, 8, 128).transpose(0, 1, 4, 3, 2)
    return np.ascontiguousarray(t).reshape(B * L, E).astype(_NPBF)


def _pack_xt_fp8(xq):
    # xt[b, i*128 + p, kc*256 + ipl*128 + l] = xq[b, i*128 + l, kc*256 + ipl*128 + p]
    t = xq.reshape(B, NT, 128, 4, 2, 128).transpose(0, 1, 5, 3, 4, 2)
    return np.ascontiguousarray(t).reshape(B * L, E)


def _pack_w_bf16(Wt):
    # w[p, (mh*8 + kt)*384 + m] = W.T[kt*128 + p, mcol(mh, m)]
    t = Wt.reshape(8, 128, E)[:, :, _MCOLS]          # [kt, p, mh*384+m]
    t = t.reshape(8, 128, 2, 384).transpose(1, 2, 0, 3)
    return np.ascontiguousarray(t).reshape(128, 6144).astype(_NPBF)


def _pack_w_fp8(Wq):
    # w[p, ((mh*4 + kc)*2 + ipl)*384 + m] = Wq[kc*256 + ipl*128 + p, mcol(mh, m)]
    t = Wq.reshape(4, 2, 128, E)[:, :, :, _MCOLS]    # [kc, ipl, p, mh*384+m]
    t = t.reshape(4, 2, 128, 2, 384).transpose(2, 3, 0, 1, 4)
    return np.ascontiguousarray(t).reshape(128, 6144)


def _pack_wo_fp8(Wq):
    # Wq: [NBH*128, E] rows = banded-head-major features (bi, p).
    # wo[p, c*2048 + j*1024 + e] = Wq[(c*2 + j)*128 + p, e]
    t = Wq.reshape(3, 2, 128, E).transpose(2, 0, 1, 3)
    return np.ascontiguousarray(t).reshape(128, NBH * E)


def _split_f8(a):
    hi = a.astype(_NPF8)
    lo = (a - hi.astype(np.float32)).astype(_NPF8)
    return hi, lo


class _Runner:
    """Builds the Bass program once and caches a jitted shard_map executable
    (one batch element per NeuronCore)."""

    def __init__(self):
        import jax
        from jax.sharding import Mesh, PartitionSpec
        from jax.experimental.shard_map import shard_map

        self.jax = jax
        _b2j.install_neuronx_cc_hook()
        nc = _build_program()
        self.nc = nc
        self.a_tab_np = _attn_tables().astype(_NPBF)

        partition_name = (
            nc.partition_id_tensor.name if nc.partition_id_tensor else None
        )
        in_names = []
        out_names = []
        out_avals = []
        for alloc in nc.m.functions[0].allocations:
            if not isinstance(alloc, mybir.MemoryLocationSet):
                continue
            name = alloc.memorylocations[0].name
            if alloc.kind == "ExternalInput":
                if name != partition_name:
                    in_names.append(name)
            elif alloc.kind == "ExternalOutput":
                out_names.append(name)
                out_avals.append(
                    jax.core.ShapedArray(
                        tuple(alloc.tensor_shape), mybir.dt.np(alloc.dtype)
                    )
                )
        self.in_names = in_names
        self.out_names = out_names
        self.out_avals = out_avals
        n_params = len(in_names)
        n_outs = len(out_names)
        all_names = tuple(in_names) + tuple(out_names)
        if partition_name is not None:
            all_names = all_names + (partition_name,)

        def _body(*args):
            operands = list(args)
            if partition_name is not None:
                operands.append(_b2j.partition_id_tensor())
            outs = _b2j._bass_exec_p.bind(
                *operands,
                out_avals=tuple(out_avals),
                in_names=all_names,
                out_names=tuple(out_names),
                lowering_input_output_aliases=(),
                sim_require_finite=True,
                sim_require_nnan=True,
                nc=nc,
            )
            return tuple(outs)

        devices = jax.devices()[:B]
        assert len(devices) == B
        self.mesh = Mesh(np.asarray(devices), ("core",))
        in_specs = (PartitionSpec("core"),) * (n_params + n_outs)
        out_specs = (PartitionSpec("core"),) * n_outs
        self.sharded = jax.jit(
            shard_map(
                _body,
                mesh=self.mesh,
                in_specs=in_specs,
                out_specs=out_specs,
                check_rep=False,
            ),
            donate_argnums=tuple(range(n_params, n_params + n_outs)),
            keep_unused=True,
        )

    def run_device(self, dev_args):
        jnp = self.jax.numpy
        zeros = [
            jnp.zeros((B * av.shape[0], *av.shape[1:]), av.dtype)
            for av in self.out_avals
        ]
        return self.sharded(*dev_args, *zeros)

    def prepare_inputs(self, x, W_in, W_out):
        jax = self.jax
        dev = {}
        if GEMM1_FP8:
            xh, xl = _split_f8(x)
            dev["xt8"] = np.concatenate(
                [_pack_xt_fp8(xh), _pack_xt_fp8(xl)], axis=1
            )
            Wt = np.ascontiguousarray(W_in.T) * np.float32(W_SCALE)
            Wh, Wl = _split_f8(Wt)
            dev["w_in_hi"] = np.concatenate([_pack_w_fp8(Wh)] * B, axis=0)
            dev["w_in_lo"] = np.concatenate([_pack_w_fp8(Wl)] * B, axis=0)
        else:
            dev["xt"] = _pack_xt_bf16(x)
            w_in_b = _pack_w_bf16(np.ascontiguousarray(W_in.T))
            dev["w_in"] = np.concatenate([w_in_b] * B, axis=0)
        if GEMM2_FP8:
            rows = np.concatenate(
                [np.arange(h * 128, (h + 1) * 128) for h in BANDED_HEADS]
            )
            Wq = np.ascontiguousarray(W_out.T[rows, :]) * np.float32(W_SCALE)
            Wh, Wl = _split_f8(Wq)
            dev["w_out_hi"] = np.concatenate([_pack_wo_fp8(Wh)] * B, axis=0)
            dev["w_out_lo"] = np.concatenate([_pack_wo_fp8(Wl)] * B, axis=0)
        else:
            w_out_b = _pack_w_bf16(np.ascontiguousarray(W_out.T))
            dev["w_out"] = np.concatenate([w_out_b] * B, axis=0)
        dev["a_tab"] = np.concatenate([self.a_tab_np] * B, axis=0)
        dev["r34t"] = _host_r34(x, W_in, W_out).reshape(B * 128, 8)
        return [jax.device_put(dev[name]) for name in self.in_names]

    def __call__(self, x, W_in, W_out):
        args = self.prepare_inputs(x, W_in, W_out)
        outs = self.run_device(args)
        outT = np.asarray(outs[self.out_names.index("outT")])
        # outT: [B*E, L] bf16 -> [B, L, E] fp32
        return np.ascontiguousarray(
            outT.reshape(B, E, L).transpose(0, 2, 1)
        ).astype(np.float32)


_CACHE = {}


def _get_runner() -> _Runner:
    if "runner" not in _CACHE:
        _CACHE["runner"] = _Runner()
    return _CACHE["runner"]


def kernel(x, W_in, W_out):
    x = np.ascontiguousarray(np.asarray(x, dtype=np.float32))
    W_in = np.ascontiguousarray(np.asarray(W_in, dtype=np.float32))
    W_out = np.ascontiguousarray(np.asarray(W_out, dtype=np.float32))
    assert x.shape == (B, L, E)
    return _get_runner()(x, W_in, W_out)


if __name__ == "__main__":
    rng = np.random.default_rng(0)
    x = rng.standard_normal((B, L, E), dtype=np.float32)
    W_in = rng.standard_normal((E, E), dtype=np.float32) * 0.05
    W_out = rng.standard_normal((E, E), dtype=np.float32) * 0.05
    y = kernel(x, W_in, W_out)
    print("out", y.shape, y.dtype, np.abs(y).mean())
